# revision 1
# baseline (speedup 1.0000x reference)
"""Trainium2 Bass kernel for a 2-layer GCN (GCNConv -> ReLU -> Linear).

Math (matching the PyG-style reference):
    deg  = in_degree(dst) + 1 (self loops), dinv = deg^-1/2
    h    = X @ W1                                  [N, 64]
    agg[d] = dinv[d] * sum_{e:(s->d)} dinv[s]*h[s] (+ self loop)   [N, 64]
    out  = relu(agg + b1) @ Wfc.T + bfc            [N, 40]

Distribution over 8 NeuronCores (graph/data parallel):
  - Nodes are re-labeled into 392 "tiles" of 128 slots (balanced by degree),
    49 tiles per core.  Each core computes h' = dinv*h for its 6272 slots
    (X @ W1 on the tensor engine), writes them as bf16 rows padded to 256 B,
    and an AllGather replicates the full 50176-row table into every core's
    HBM.
  - Each core aggregates the edges whose destination it owns: a bulk
    SWDGE dma_gather fetches h'[src] rows (256 B each) into SBUF in
    edge-major layout; destinations are scatter-added via one-hot matmuls
    (lhsT = gathered messages [128 edges, 64], rhs = C [128 edges, 128 dst]
    with C[j, d] = dinv_dst[j] * (dst_loc[j] == d)) accumulated in PSUM.
    C is built on the vector engine with a single fused tensor_scalar
    (is_equal then mult) per 128-edge chunk.
  - relu(agg + b1) is fused into the PSUM->SBUF eviction on the scalar
    engine (per-partition bias = b1 since agg is kept feature-major), the
    second layer is one small matmul per tile, and the bfc bias rides the
    final eviction.  The host un-permutes the [40, slots] outputs.

dma_gather indices are int16, so the row table is split at row 32768 into a
"lo" and a "hi" region, and each tile's edges are packed into K_LO lo-chunks
followed by K_HI hi-chunks (pad lanes: idx=0, dst_loc=-1, dinv_dst=0).
"""

import numpy as np

# ----------------------------------------------------------------------------
# Problem configuration (hardcoded; kernel.py must be self-contained).
# ----------------------------------------------------------------------------
N_NODES = 50000
N_EDGES = 800000
IN_DIM = 512
HID = 64
OUT_DIM = 40
N_CORES = 8

# Gather-call shape: ≤GATHER_MAX_CHUNKS*128 indices per dma_gather call.
# single_packet=True is limited to 64 descriptors (1024 idxs) per call and
# serializes each call onto one SDMA engine.
GATHER_MAX_CHUNKS = 18
GATHER_SINGLE_PACKET = False


class Cfg:
    def __init__(self, n_nodes, in_dim, hid, out_dim, n_cores, tiles_per_core,
                 group, lo_boundary, k_lo, k_hi):
        self.n_nodes = n_nodes
        self.in_dim = in_dim
        self.hid = hid
        self.out_dim = out_dim
        self.n_cores = n_cores
        self.nt = tiles_per_core              # tiles per core
        self.group = group                    # tiles per gather group
        assert self.nt % self.group == 0
        self.n_groups = self.nt // self.group
        self.slots_per_core = self.nt * 128
        self.n_tiles = n_cores * self.nt
        self.n_slots = self.n_tiles * 128
        self.lo_b = lo_boundary               # table split row (<= 32768)
        self.k_lo = k_lo                      # lo chunks per tile
        self.k_hi = k_hi                      # hi chunks per tile
        self.k = k_lo + k_hi
        self.kd = in_dim // 128               # contraction tiles for X @ W1
        assert in_dim % 128 == 0
        assert self.n_slots - self.lo_b <= 32768 and self.lo_b <= 32768


# ----------------------------------------------------------------------------
# Host-side graph preprocessing (index/layout work only; all feature math
# runs on the device).
# ----------------------------------------------------------------------------

def _plan(edges, cfg_base):
    """Relabel nodes into balanced tiles and pack edges into chunk slots.

    Returns (cfg, plan dict).  cfg_base is (n_nodes, in_dim, hid, out_dim,
    n_cores, nt, group, lo_boundary); k_lo/k_hi are derived from the data.
    """
    n_nodes, in_dim, hid, out_dim, n_cores, nt, group, lo_b = cfg_base
    n_tiles = n_cores * nt
    n_slots = n_tiles * 128

    src = np.asarray(edges[0], dtype=np.int64)
    dst = np.asarray(edges[1], dtype=np.int64)
    deg = np.bincount(dst, minlength=n_nodes).astype(np.int64) + 1
    dinv = (1.0 / np.sqrt(deg.astype(np.float64))).astype(np.float32)

    # Snake binpack nodes (by degree, desc) into n_tiles bins of <=128 slots.
    order = np.argsort(-deg, kind="stable")
    rounds = np.arange(n_nodes) // n_tiles
    pos = np.arange(n_nodes) % n_tiles
    tile_of = np.where(rounds % 2 == 0, pos, n_tiles - 1 - pos)
    assert rounds.max() < 128, "more than 128 slots per tile"
    node_to_slot = np.empty(n_nodes, dtype=np.int64)
    node_to_slot[order] = tile_of * 128 + rounds

    slot_dinv = np.zeros(n_slots, dtype=np.float32)
    slot_dinv[node_to_slot] = dinv

    # Full edge list including self loops, in slot space.
    s_slot = np.concatenate([node_to_slot[src], node_to_slot])
    d_slot = np.concatenate([node_to_slot[dst], node_to_slot])
    d_tile = d_slot >> 7
    is_hi = (s_slot >= lo_b).astype(np.int64)

    # Group edges by (dst tile, lo/hi class).
    key = d_tile * 2 + is_hi
    sort_idx = np.argsort(key, kind="stable")
    key_s = key[sort_idx]
    s_slot_s = s_slot[sort_idx]
    d_slot_s = d_slot[sort_idx]
    counts = np.bincount(key_s, minlength=n_tiles * 2)
    starts = np.concatenate([[0], np.cumsum(counts)[:-1]])
    rank_in_group = np.arange(len(key_s)) - starts[key_s]

    n_lo = counts[0::2]
    n_hi = counts[1::2]
    k_lo = int(np.max((n_lo + 127) // 128)) if n_lo.max() > 0 else 0
    k_hi = int(np.max((n_hi + 127) // 128)) if n_hi.max() > 0 else 0
    k_lo = max(k_lo, 1)
    k_hi = max(k_hi, 1)

    cfg = Cfg(n_nodes, in_dim, hid, out_dim, n_cores, nt, group, lo_b,
              k_lo, k_hi)

    # Per-core chunk-slot numbering (group-major, lo chunks then hi chunks
    # inside each group):
    #   lo: fc = g*(G*K) + i*K_LO + j
    #   hi: fc = g*(G*K) + G*K_LO + i*K_HI + j
    n_chunks_core = nt * cfg.k
    g_of_tile = (d_tile % nt) // group        # group within core
    i_of_tile = (d_tile % nt) % group         # tile within group
    core_of = d_tile // nt
    j_chunk = rank_in_group >> 7
    lane = rank_in_group & 127
    base = g_of_tile[sort_idx] * (group * cfg.k)
    fc = np.where(
        key_s % 2 == 0,
        base + i_of_tile[sort_idx] * k_lo + j_chunk,
        base + group * k_lo + i_of_tile[sort_idx] * k_hi + j_chunk,
    )
    assert (j_chunk < np.where(key_s % 2 == 0, k_lo, k_hi)).all()

    idx16 = np.zeros((n_cores, n_chunks_core, 128), dtype=np.int16)
    dstloc = np.full((n_cores, n_chunks_core, 128), -1.0, dtype=np.float32)
    dinvdst = np.zeros((n_cores, n_chunks_core, 128), dtype=np.float32)

    cidx = core_of[sort_idx]
    idx16[cidx, fc, lane] = np.where(
        key_s % 2 == 0, s_slot_s, s_slot_s - lo_b).astype(np.int16)
    dstloc[cidx, fc, lane] = (d_slot_s & 127).astype(np.float32)
    dinvdst[cidx, fc, lane] = slot_dinv[d_slot_s]

    # Wrap gather indices: per (group, class) region, list position s ->
    # partition s%16, column s//16; replicated across the 8 q7 cores
    # (128 partitions).
    n_idx_cols = n_chunks_core * 128 // 16
    idx_wrapped = np.zeros((n_cores, 128, n_idx_cols), dtype=np.int16)
    for g in range(cfg.n_groups):
        for cls in range(2):
            fc0 = g * group * cfg.k + (0 if cls == 0 else group * k_lo)
            nch = group * (k_lo if cls == 0 else k_hi)
            flat = idx16[:, fc0:fc0 + nch, :].reshape(n_cores, nch * 128)
            wrapped = flat.reshape(n_cores, nch * 8, 16).transpose(0, 2, 1)
            c0 = fc0 * 8
            idx_wrapped[:, :16, c0:c0 + nch * 8] = wrapped
    idx_wrapped[:, 16:, :] = np.tile(idx_wrapped[:, :16, :], (1, 7, 1))

    plan = dict(
        node_to_slot=node_to_slot,
        slot_dinv=slot_dinv,
        idx_wrapped=idx_wrapped,
        dstloc=dstloc.transpose(0, 2, 1).copy(),    # [cores, 128, n_chunks]
        dinvdst=dinvdst.transpose(0, 2, 1).copy(),
    )
    return cfg, plan


def _make_inputs(X, W1, b1, Wfc, bfc, cfg, plan):
    """Build the 8 per-core input dicts for run_bass_kernel_spmd."""
    import ml_dtypes
    bf16 = ml_dtypes.bfloat16
    node_to_slot = plan["node_to_slot"]
    s = cfg.slots_per_core

    Xp = np.zeros((cfg.n_slots, cfg.in_dim), dtype=np.float32)
    Xp[node_to_slot] = np.asarray(X, dtype=np.float32)

    W1r = (np.asarray(W1, dtype=np.float32)
           .reshape(cfg.kd, 128, cfg.hid).transpose(1, 0, 2)
           .reshape(128, cfg.kd * cfg.hid).astype(bf16))
    wfcT = np.ascontiguousarray(np.asarray(Wfc, dtype=np.float32).T).astype(bf16)
    b1c = np.asarray(b1, dtype=np.float32).reshape(cfg.hid, 1)
    bfcc = np.asarray(bfc, dtype=np.float32).reshape(cfg.out_dim, 1)
    iota = np.tile(np.arange(128, dtype=np.float32), (128, 1)).astype(bf16)

    in_maps = []
    for c in range(cfg.n_cores):
        xt = np.ascontiguousarray(Xp[c * s:(c + 1) * s].T).astype(bf16)
        dinv_sb = np.ascontiguousarray(
            plan["slot_dinv"][c * s:(c + 1) * s].reshape(cfg.nt, 128).T)
        in_maps.append({
            "xt": xt,
            "w1": W1r,
            "wfcT": wfcT,
            "b1": b1c,
            "bfc": bfcc,
            "iota": iota,
            "dinv_sb": dinv_sb,
            "idx": plan["idx_wrapped"][c],
            "dstloc": plan["dstloc"][c],
            "dinvdst": plan["dinvdst"][c],
        })
    return in_maps


# ----------------------------------------------------------------------------
# Device kernel.
# ----------------------------------------------------------------------------

def _build_module(cfg):
    import concourse.bass as bass
    import concourse.bacc as bacc
    import concourse.mybir as mybir
    import concourse.tile as tile
    from contextlib import ExitStack

    f32 = mybir.dt.float32
    bf16 = mybir.dt.bfloat16
    i16 = mybir.dt.int16
    S = cfg.slots_per_core
    G = cfg.group
    NCHG = G * cfg.k                      # chunks per group
    GKLO = G * cfg.k_lo                   # lo chunks per group
    n_chunks = cfg.nt * cfg.k
    n_idx_cols = n_chunks * 128 // 16

    nc = bacc.Bacc("TRN2", target_bir_lowering=False, debug=False,
                   num_devices=cfg.n_cores)

    xt_d = nc.dram_tensor("xt", [cfg.in_dim, S], bf16, kind="ExternalInput")
    w1_d = nc.dram_tensor("w1", [128, cfg.kd * cfg.hid], bf16,
                          kind="ExternalInput")
    wfcT_d = nc.dram_tensor("wfcT", [cfg.hid, cfg.out_dim], bf16,
                            kind="ExternalInput")
    b1_d = nc.dram_tensor("b1", [cfg.hid, 1], f32, kind="ExternalInput")
    bfc_d = nc.dram_tensor("bfc", [cfg.out_dim, 1], f32, kind="ExternalInput")
    iota_d = nc.dram_tensor("iota", [128, 128], bf16, kind="ExternalInput")
    dinv_d = nc.dram_tensor("dinv_sb", [128, cfg.nt], f32,
                            kind="ExternalInput")
    idx_d = nc.dram_tensor("idx", [128, n_idx_cols], i16, kind="ExternalInput")
    dstloc_d = nc.dram_tensor("dstloc", [128, n_chunks], f32,
                              kind="ExternalInput")
    dinvdst_d = nc.dram_tensor("dinvdst", [128, n_chunks], f32,
                               kind="ExternalInput")
    out_d = nc.dram_tensor("out", [cfg.out_dim, S], f32, kind="ExternalOutput")

    with tile.TileContext(nc) as tc, ExitStack() as ctx:
        dram = ctx.enter_context(tc.tile_pool(name="dram", bufs=1,
                                              space="DRAM"))
        consts = ctx.enter_context(tc.tile_pool(name="consts", bufs=1))
        ag_in = dram.tile([S, 128], bf16)
        ag_out = dram.tile([cfg.n_slots, 128], bf16)

        iota_sb = consts.tile([128, 128], bf16)
        w1_sb = consts.tile([128, cfg.kd * cfg.hid], bf16)
        wfcT_sb = consts.tile([cfg.hid, cfg.out_dim], bf16)
        b1_sb = consts.tile([cfg.hid, 1], f32)
        bfc_sb = consts.tile([cfg.out_dim, 1], f32)
        dinv_sb = consts.tile([128, cfg.nt], f32)
        idx_sb = consts.tile([128, n_idx_cols], i16)
        dstloc_sb = consts.tile([128, n_chunks], f32)
        dinvdst_sb = consts.tile([128, n_chunks], f32)

        nc.sync.dma_start(iota_sb[:], iota_d[:])
        nc.sync.dma_start(w1_sb[:], w1_d[:])
        nc.sync.dma_start(wfcT_sb[:], wfcT_d[:])
        nc.sync.dma_start(b1_sb[:], b1_d[:])
        nc.sync.dma_start(bfc_sb[:], bfc_d[:])
        nc.sync.dma_start(dinv_sb[:], dinv_d[:])
        nc.sync.dma_start(idx_sb[:], idx_d[:])
        nc.sync.dma_start(dstloc_sb[:], dstloc_d[:])
        nc.sync.dma_start(dinvdst_sb[:], dinvdst_d[:])

        # ---- Phase 1: h' = dinv * (X @ W1), bf16 rows padded to 256 B ----
        with tc.tile_pool(name="p1", bufs=1) as p1, \
                tc.tile_pool(name="p1ps", bufs=2, space="PSUM") as p1ps:
            xt_sb = p1.tile([128, cfg.kd, S], bf16)
            stage = p1.tile([128, cfg.nt, 128], bf16)
            nc.sync.dma_start(
                xt_sb[:],
                xt_d[:].rearrange("(k p) s -> p k s", p=128))
            nc.vector.memset(stage[:], 0.0)
            for t in range(cfg.nt):
                ph = p1ps.tile([128, cfg.hid], f32)
                for k in range(cfg.kd):
                    nc.tensor.matmul(
                        ph[:],
                        xt_sb[:, k, t * 128:(t + 1) * 128],
                        w1_sb[:, k * cfg.hid:(k + 1) * cfg.hid],
                        start=(k == 0), stop=(k == cfg.kd - 1))
                nc.vector.tensor_scalar_mul(
                    stage[:, t, 0:cfg.hid], ph[:],
                    dinv_sb[:, t:t + 1])
            nc.sync.dma_start(
                ag_in[:].rearrange("(t p) e -> p t e", p=128), stage[:])

        # ---- AllGather the h' table across all cores ----
        nc.gpsimd.collective_compute(
            "AllGather",
            mybir.AluOpType.bypass,
            ins=[ag_in.opt()],
            outs=[ag_out.opt()],
            replica_groups=[list(range(cfg.n_cores))],
        )

        # ---- Phase 2: gather + one-hot scatter matmuls + layer 2 ----
        msgs_p = ctx.enter_context(tc.tile_pool(name="msgs", bufs=2))
        c_p = ctx.enter_context(tc.tile_pool(name="cmat", bufs=8))
        relu_p = ctx.enter_context(tc.tile_pool(name="relu", bufs=3))
        ost_p = ctx.enter_context(tc.tile_pool(name="ost", bufs=2))
        agg_ps = ctx.enter_context(
            tc.tile_pool(name="aggps", bufs=4, space="PSUM"))
        o2_ps = ctx.enter_context(
            tc.tile_pool(name="o2ps", bufs=2, space="PSUM"))

        GMAX = GATHER_MAX_CHUNKS   # max chunks per gather call

        for g in range(cfg.n_groups):
            msgs = msgs_p.tile([128, NCHG, 128], bf16)
            col0 = g * NCHG * 8
            for r0, r1, tbl in ((0, GKLO, ag_out[0:cfg.lo_b, :]),
                                (GKLO, NCHG,
                                 ag_out[cfg.lo_b:cfg.n_slots, :])):
                cs0 = r0
                while cs0 < r1:
                    nch = min(GMAX, r1 - cs0)
                    nc.gpsimd.dma_gather(
                        msgs[:, cs0:cs0 + nch, :], tbl,
                        idx_sb[:, col0 + cs0 * 8: col0 + (cs0 + nch) * 8],
                        nch * 128, nch * 128, 128,
                        single_packet=GATHER_SINGLE_PACKET)
                    cs0 += nch

            for i in range(G):
                t = g * G + i
                agg = agg_ps.tile([cfg.hid, 128], f32)
                slots = ([i * cfg.k_lo + j for j in range(cfg.k_lo)]
                         + [GKLO + i * cfg.k_hi + j for j in range(cfg.k_hi)])
                for jj, cs in enumerate(slots):
                    gc = g * NCHG + cs
                    cmat = c_p.tile([128, 128], bf16)
                    nc.vector.tensor_scalar(
                        cmat[:], iota_sb[:],
                        dstloc_sb[:, gc:gc + 1],
                        dinvdst_sb[:, gc:gc + 1],
                        mybir.AluOpType.is_equal,
                        mybir.AluOpType.mult)
                    nc.tensor.matmul(
                        agg[:], msgs[:, cs, 0:cfg.hid], cmat[:],
                        start=(jj == 0), stop=(jj == len(slots) - 1))
                relu = relu_p.tile([cfg.hid, 128], bf16)
                nc.scalar.activation(
                    relu[:], agg[:], mybir.ActivationFunctionType.Relu,
                    bias=b1_sb[:])
                o2 = o2_ps.tile([cfg.out_dim, 128], f32)
                nc.tensor.matmul(o2[:], wfcT_sb[:], relu[:],
                                 start=True, stop=True)
                if i == 0:
                    ostage = ost_p.tile([cfg.out_dim, G * 128], f32)
                nc.scalar.activation(
                    ostage[:, i * 128:(i + 1) * 128], o2[:],
                    mybir.ActivationFunctionType.Identity, bias=bfc_sb[:])
            nc.sync.dma_start(
                out_d[:, g * G * 128:(g + 1) * G * 128], ostage[:])

    nc.compile()
    return nc


# ----------------------------------------------------------------------------
# Entry points.
# ----------------------------------------------------------------------------

_CACHE = {}


def _get_compiled(edges, cfg_base):
    import hashlib
    e = np.ascontiguousarray(np.asarray(edges, dtype=np.int64))
    key = (e.shape, hashlib.sha1(e.tobytes()).hexdigest(), cfg_base)
    if key not in _CACHE:
        cfg, plan = _plan(e, cfg_base)
        nc = _build_module(cfg)
        _CACHE[key] = (cfg, plan, nc)
    return _CACHE[key]


def _run(X, edges, W1, b1, Wfc, bfc, cfg_base, trace=False):
    from concourse.bass_utils import run_bass_kernel_spmd

    cfg, plan, nc = _get_compiled(edges, cfg_base)
    in_maps = _make_inputs(X, W1, b1, Wfc, bfc, cfg, plan)
    res = run_bass_kernel_spmd(
        nc, in_maps, core_ids=list(range(cfg.n_cores)), trace=trace)

    s = cfg.slots_per_core
    full = np.concatenate([res.results[c]["out"] for c in range(cfg.n_cores)],
                          axis=1)                      # [40, n_slots]
    out = full[:, plan["node_to_slot"]].T.astype(np.float32)
    out = np.ascontiguousarray(out)
    return out, res


def kernel(X, edges, W1, b1, Wfc, bfc):
    cfg_base = (N_NODES, IN_DIM, HID, OUT_DIM, N_CORES, 49, 7, 32768)
    out, _ = _run(np.asarray(X, dtype=np.float32), np.asarray(edges),
                  np.asarray(W1, dtype=np.float32),
                  np.asarray(b1, dtype=np.float32),
                  np.asarray(Wfc, dtype=np.float32),
                  np.asarray(bfc, dtype=np.float32), cfg_base)
    return out



# revision 6
# speedup vs baseline: 1.0610x; 1.0610x over previous
"""Trainium2 Bass kernel for a 2-layer GCN (GCNConv -> ReLU -> Linear).

Math (matching the PyG-style reference):
    deg  = in_degree(dst) + 1 (self loops), dinv = deg^-1/2
    h    = X @ W1                                  [N, 64]
    agg[d] = dinv[d] * sum_{e:(s->d)} dinv[s]*h[s] (+ self loop)   [N, 64]
    out  = relu(agg + b1) @ Wfc.T + bfc            [N, 40]

Distribution over 8 NeuronCores (graph/data parallel):
  - Nodes are re-labeled into 392 "tiles" of 128 slots (balanced by degree),
    49 tiles per core.  Each core computes h' = dinv*h for its 6272 slots
    (X @ W1 on the tensor engine), writes them as bf16 rows padded to 256 B,
    and an AllGather replicates the full 50176-row table into every core's
    HBM.
  - Each core aggregates the edges whose destination it owns: a bulk
    SWDGE dma_gather fetches h'[src] rows (256 B each) into SBUF in
    edge-major layout; destinations are scatter-added via one-hot matmuls
    (lhsT = gathered messages [128 edges, 64], rhs = C [128 edges, 128 dst]
    with C[j, d] = dinv_dst[j] * (dst_loc[j] == d)) accumulated in PSUM.
    C is built on the vector engine with a single fused tensor_scalar
    (is_equal then mult) per 128-edge chunk.
  - relu(agg + b1) is fused into the PSUM->SBUF eviction on the scalar
    engine (per-partition bias = b1 since agg is kept feature-major), the
    second layer is one small matmul per tile, and the bfc bias rides the
    final eviction.  The host un-permutes the [40, slots] outputs.

dma_gather indices are int16, so the row table is split at row 32768 into a
"lo" and a "hi" region, and each tile's edges are packed into K_LO lo-chunks
followed by K_HI hi-chunks (pad lanes: idx=0, dst_loc=-1, dinv_dst=0).
"""

import numpy as np

# ----------------------------------------------------------------------------
# Problem configuration (hardcoded; kernel.py must be self-contained).
# ----------------------------------------------------------------------------
N_NODES = 50000
N_EDGES = 800000
IN_DIM = 512
HID = 64
OUT_DIM = 40
N_CORES = 8

# Gather-call shape: ≤GATHER_MAX_CHUNKS*128 indices per dma_gather call.
# single_packet=True is limited to 64 descriptors (1024 idxs) per call and
# serializes each call onto one SDMA engine.
GATHER_MAX_CHUNKS = 18
GATHER_SINGLE_PACKET = False


class Cfg:
    def __init__(self, n_nodes, in_dim, hid, out_dim, n_cores, tiles_per_core,
                 group, lo_boundary, k_lo, k_hi):
        self.n_nodes = n_nodes
        self.in_dim = in_dim
        self.hid = hid
        self.out_dim = out_dim
        self.n_cores = n_cores
        self.nt = tiles_per_core              # tiles per core
        self.group = group                    # tiles per gather group
        assert self.nt % self.group == 0
        self.n_groups = self.nt // self.group
        self.slots_per_core = self.nt * 128
        self.n_tiles = n_cores * self.nt
        self.n_slots = self.n_tiles * 128
        self.lo_b = lo_boundary               # table split row (<= 32768)
        self.k_lo = k_lo                      # lo chunks per tile
        self.k_hi = k_hi                      # hi chunks per tile
        self.k = k_lo + k_hi
        self.kd = in_dim // 128               # contraction tiles for X @ W1
        assert in_dim % 128 == 0
        assert self.n_slots - self.lo_b <= 32768 and self.lo_b <= 32768


# ----------------------------------------------------------------------------
# Host-side graph preprocessing (index/layout work only; all feature math
# runs on the device).
# ----------------------------------------------------------------------------

def _plan(edges, cfg_base):
    """Relabel nodes into balanced tiles and pack edges into chunk slots.

    Returns (cfg, plan dict).  cfg_base is (n_nodes, in_dim, hid, out_dim,
    n_cores, nt, group, lo_boundary); k_lo/k_hi are derived from the data.
    """
    n_nodes, in_dim, hid, out_dim, n_cores, nt, group, lo_b = cfg_base
    n_tiles = n_cores * nt
    n_slots = n_tiles * 128

    src = np.asarray(edges[0], dtype=np.int64)
    dst = np.asarray(edges[1], dtype=np.int64)
    deg = np.bincount(dst, minlength=n_nodes).astype(np.int64) + 1
    dinv = (1.0 / np.sqrt(deg.astype(np.float64))).astype(np.float32)

    # Snake binpack nodes (by degree, desc) into n_tiles bins of <=128 slots.
    order = np.argsort(-deg, kind="stable")
    rounds = np.arange(n_nodes) // n_tiles
    pos = np.arange(n_nodes) % n_tiles
    tile_of = np.where(rounds % 2 == 0, pos, n_tiles - 1 - pos)
    assert rounds.max() < 128, "more than 128 slots per tile"
    node_to_slot = np.empty(n_nodes, dtype=np.int64)
    node_to_slot[order] = tile_of * 128 + rounds

    slot_dinv = np.zeros(n_slots, dtype=np.float32)
    slot_dinv[node_to_slot] = dinv

    # Full edge list including self loops, in slot space.
    s_slot = np.concatenate([node_to_slot[src], node_to_slot])
    d_slot = np.concatenate([node_to_slot[dst], node_to_slot])
    d_tile = d_slot >> 7
    is_hi = (s_slot >= lo_b).astype(np.int64)

    # Group edges by (dst tile, lo/hi class).
    key = d_tile * 2 + is_hi
    sort_idx = np.argsort(key, kind="stable")
    key_s = key[sort_idx]
    s_slot_s = s_slot[sort_idx]
    d_slot_s = d_slot[sort_idx]
    counts = np.bincount(key_s, minlength=n_tiles * 2)
    starts = np.concatenate([[0], np.cumsum(counts)[:-1]])
    rank_in_group = np.arange(len(key_s)) - starts[key_s]

    n_lo = counts[0::2]
    n_hi = counts[1::2]
    k_lo = int(np.max((n_lo + 127) // 128)) if n_lo.max() > 0 else 0
    k_hi = int(np.max((n_hi + 127) // 128)) if n_hi.max() > 0 else 0
    k_lo = max(k_lo, 1)
    k_hi = max(k_hi, 1)

    cfg = Cfg(n_nodes, in_dim, hid, out_dim, n_cores, nt, group, lo_b,
              k_lo, k_hi)

    # Per-core chunk-slot numbering (group-major, lo chunks then hi chunks
    # inside each group):
    #   lo: fc = g*(G*K) + i*K_LO + j
    #   hi: fc = g*(G*K) + G*K_LO + i*K_HI + j
    n_chunks_core = nt * cfg.k
    g_of_tile = (d_tile % nt) // group        # group within core
    i_of_tile = (d_tile % nt) % group         # tile within group
    core_of = d_tile // nt
    j_chunk = rank_in_group >> 7
    lane = rank_in_group & 127
    base = g_of_tile[sort_idx] * (group * cfg.k)
    fc = np.where(
        key_s % 2 == 0,
        base + i_of_tile[sort_idx] * k_lo + j_chunk,
        base + group * k_lo + i_of_tile[sort_idx] * k_hi + j_chunk,
    )
    assert (j_chunk < np.where(key_s % 2 == 0, k_lo, k_hi)).all()

    idx16 = np.zeros((n_cores, n_chunks_core, 128), dtype=np.int16)
    dstloc = np.full((n_cores, n_chunks_core, 128), -1.0, dtype=np.float32)
    dinvdst = np.zeros((n_cores, n_chunks_core, 128), dtype=np.float32)

    cidx = core_of[sort_idx]
    idx16[cidx, fc, lane] = np.where(
        key_s % 2 == 0, s_slot_s, s_slot_s - lo_b).astype(np.int16)
    dstloc[cidx, fc, lane] = (d_slot_s & 127).astype(np.float32)
    dinvdst[cidx, fc, lane] = slot_dinv[d_slot_s]

    # Wrap gather indices: per (group, class) region, list position s ->
    # partition s%16, column s//16; replicated across the 8 q7 cores
    # (128 partitions).
    n_idx_cols = n_chunks_core * 128 // 16
    idx_wrapped = np.zeros((n_cores, 128, n_idx_cols), dtype=np.int16)
    for g in range(cfg.n_groups):
        for cls in range(2):
            fc0 = g * group * cfg.k + (0 if cls == 0 else group * k_lo)
            nch = group * (k_lo if cls == 0 else k_hi)
            flat = idx16[:, fc0:fc0 + nch, :].reshape(n_cores, nch * 128)
            wrapped = flat.reshape(n_cores, nch * 8, 16).transpose(0, 2, 1)
            c0 = fc0 * 8
            idx_wrapped[:, :16, c0:c0 + nch * 8] = wrapped
    idx_wrapped[:, 16:, :] = np.tile(idx_wrapped[:, :16, :], (1, 7, 1))

    plan = dict(
        node_to_slot=node_to_slot,
        slot_dinv=slot_dinv,
        idx_wrapped=idx_wrapped,
        dstloc=dstloc.transpose(0, 2, 1).copy(),    # [cores, 128, n_chunks]
        dinvdst=dinvdst.transpose(0, 2, 1).copy(),
    )
    return cfg, plan


def _make_inputs(X, W1, b1, Wfc, bfc, cfg, plan):
    """Build the 8 per-core input dicts for run_bass_kernel_spmd."""
    import ml_dtypes
    bf16 = ml_dtypes.bfloat16
    node_to_slot = plan["node_to_slot"]
    s = cfg.slots_per_core

    Xp = np.zeros((cfg.n_slots, cfg.in_dim), dtype=np.float32)
    Xp[node_to_slot] = np.asarray(X, dtype=np.float32)

    W1r = (np.asarray(W1, dtype=np.float32)
           .reshape(cfg.kd, 128, cfg.hid).transpose(1, 0, 2)
           .reshape(128, cfg.kd * cfg.hid).astype(bf16))
    wfcT = np.ascontiguousarray(np.asarray(Wfc, dtype=np.float32).T).astype(bf16)
    b1c = np.asarray(b1, dtype=np.float32).reshape(cfg.hid, 1)
    bfcc = np.asarray(bfc, dtype=np.float32).reshape(cfg.out_dim, 1)
    iota = np.ascontiguousarray(
        np.tile(np.arange(128, dtype=np.float32), (128, 1)))

    in_maps = []
    for c in range(cfg.n_cores):
        xt = np.ascontiguousarray(Xp[c * s:(c + 1) * s].T).astype(bf16)
        dinv_sb = np.ascontiguousarray(
            plan["slot_dinv"][c * s:(c + 1) * s].reshape(cfg.nt, 128).T)
        in_maps.append({
            "xt": xt,
            "w1": W1r,
            "wfcT": wfcT,
            "b1": b1c,
            "bfc": bfcc,
            "iota": iota,
            "dinv_sb": dinv_sb,
            "idx": plan["idx_wrapped"][c],
            "dstloc": plan["dstloc"][c],
            "dinvdst": plan["dinvdst"][c],
        })
    return in_maps


# ----------------------------------------------------------------------------
# Device kernel.
# ----------------------------------------------------------------------------

def _build_module(cfg):
    import concourse.bass as bass
    import concourse.bacc as bacc
    import concourse.mybir as mybir
    import concourse.tile as tile
    from contextlib import ExitStack

    f32 = mybir.dt.float32
    bf16 = mybir.dt.bfloat16
    i16 = mybir.dt.int16
    S = cfg.slots_per_core
    G = cfg.group
    NCHG = G * cfg.k                      # chunks per group
    GKLO = G * cfg.k_lo                   # lo chunks per group
    n_chunks = cfg.nt * cfg.k
    n_idx_cols = n_chunks * 128 // 16

    nc = bacc.Bacc("TRN2", target_bir_lowering=False, debug=False,
                   num_devices=cfg.n_cores)

    xt_d = nc.dram_tensor("xt", [cfg.in_dim, S], bf16, kind="ExternalInput")
    w1_d = nc.dram_tensor("w1", [128, cfg.kd * cfg.hid], bf16,
                          kind="ExternalInput")
    wfcT_d = nc.dram_tensor("wfcT", [cfg.hid, cfg.out_dim], bf16,
                            kind="ExternalInput")
    b1_d = nc.dram_tensor("b1", [cfg.hid, 1], f32, kind="ExternalInput")
    bfc_d = nc.dram_tensor("bfc", [cfg.out_dim, 1], f32, kind="ExternalInput")
    iota_d = nc.dram_tensor("iota", [128, 128], f32, kind="ExternalInput")
    dinv_d = nc.dram_tensor("dinv_sb", [128, cfg.nt], f32,
                            kind="ExternalInput")
    idx_d = nc.dram_tensor("idx", [128, n_idx_cols], i16, kind="ExternalInput")
    dstloc_d = nc.dram_tensor("dstloc", [128, n_chunks], f32,
                              kind="ExternalInput")
    dinvdst_d = nc.dram_tensor("dinvdst", [128, n_chunks], f32,
                               kind="ExternalInput")
    out_d = nc.dram_tensor("out", [cfg.out_dim, S], f32, kind="ExternalOutput")

    with tile.TileContext(nc) as tc, ExitStack() as ctx:
        dram = ctx.enter_context(tc.tile_pool(name="dram", bufs=1,
                                              space="DRAM"))
        consts = ctx.enter_context(tc.tile_pool(name="consts", bufs=1))
        ag_in = dram.tile([S, 128], bf16)
        ag_out = dram.tile([cfg.n_slots, 128], bf16, addr_space="Shared")

        # iota lives in PSUM: a PSUM-source tensor_scalar cannot enter the
        # 2-port DVE perf modes, so the per-chunk C-matrix builds never grab
        # the shared SBUF port pair that SWDGE descriptor generation (the
        # dma_gather Q7 loop) needs — they'd otherwise serialize.
        iota_pp = ctx.enter_context(
            tc.tile_pool(name="iotapp", bufs=1, space="PSUM"))
        iota_ps = iota_pp.tile([128, 128], f32)

        iota_sb = consts.tile([128, 128], f32)
        w1_sb = consts.tile([128, cfg.kd * cfg.hid], bf16)
        wfcT_sb = consts.tile([cfg.hid, cfg.out_dim], bf16)
        b1_sb = consts.tile([cfg.hid, 1], f32)
        bfc_sb = consts.tile([cfg.out_dim, 1], f32)
        dinv_sb = consts.tile([128, cfg.nt], f32)
        idx_sb = consts.tile([128, n_idx_cols], i16)
        dstloc_sb = consts.tile([128, n_chunks], f32)
        dinvdst_sb = consts.tile([128, n_chunks], f32)

        nc.sync.dma_start(iota_sb[:], iota_d[:])
        nc.vector.tensor_scalar_mul(iota_ps[:], iota_sb[:], 1.0)
        nc.sync.dma_start(w1_sb[:], w1_d[:])
        nc.sync.dma_start(wfcT_sb[:], wfcT_d[:])
        nc.sync.dma_start(b1_sb[:], b1_d[:])
        nc.sync.dma_start(bfc_sb[:], bfc_d[:])
        nc.sync.dma_start(dinv_sb[:], dinv_d[:])
        nc.sync.dma_start(idx_sb[:], idx_d[:])
        nc.sync.dma_start(dstloc_sb[:], dstloc_d[:])
        nc.sync.dma_start(dinvdst_sb[:], dinvdst_d[:])

        # ---- Phase 1: h' = dinv * (X @ W1), bf16 rows padded to 256 B ----
        with tc.tile_pool(name="p1", bufs=1) as p1, \
                tc.tile_pool(name="p1ps", bufs=2, space="PSUM") as p1ps:
            xt_sb = p1.tile([128, cfg.kd, S], bf16)
            stage = p1.tile([128, cfg.nt, 128], bf16)
            nc.sync.dma_start(
                xt_sb[:],
                xt_d[:].rearrange("(k p) s -> p k s", p=128))
            nc.vector.memset(stage[:], 0.0)
            for t in range(cfg.nt):
                ph = p1ps.tile([128, cfg.hid], f32)
                for k in range(cfg.kd):
                    nc.tensor.matmul(
                        ph[:],
                        xt_sb[:, k, t * 128:(t + 1) * 128],
                        w1_sb[:, k * cfg.hid:(k + 1) * cfg.hid],
                        start=(k == 0), stop=(k == cfg.kd - 1))
                nc.vector.tensor_scalar_mul(
                    stage[:, t, 0:cfg.hid], ph[:],
                    dinv_sb[:, t:t + 1])
            nc.sync.dma_start(
                ag_in[:].rearrange("(t p) e -> p t e", p=128), stage[:])

        # ---- AllGather the h' table across all cores ----
        nc.gpsimd.collective_compute(
            "AllGather",
            mybir.AluOpType.bypass,
            ins=[ag_in.opt()],
            outs=[ag_out.opt()],
            replica_groups=[list(range(cfg.n_cores))],
        )

        # ---- Phase 2: gather + one-hot scatter matmuls + layer 2 ----
        msgs_p = ctx.enter_context(tc.tile_pool(name="msgs", bufs=2))
        c_p = ctx.enter_context(tc.tile_pool(name="cmat", bufs=8))
        relu_p = ctx.enter_context(tc.tile_pool(name="relu", bufs=3))
        ost_p = ctx.enter_context(tc.tile_pool(name="ost", bufs=2))
        agg_ps = ctx.enter_context(
            tc.tile_pool(name="aggps", bufs=4, space="PSUM"))
        o2_ps = ctx.enter_context(
            tc.tile_pool(name="o2ps", bufs=2, space="PSUM"))

        GMAX = GATHER_MAX_CHUNKS   # max chunks per gather call

        for g in range(cfg.n_groups):
            msgs = msgs_p.tile([128, NCHG, 128], bf16)
            col0 = g * NCHG * 8
            for r0, r1, tbl in ((0, GKLO, ag_out[0:cfg.lo_b, :]),
                                (GKLO, NCHG,
                                 ag_out[cfg.lo_b:cfg.n_slots, :])):
                cs0 = r0
                while cs0 < r1:
                    nch = min(GMAX, r1 - cs0)
                    nc.gpsimd.dma_gather(
                        msgs[:, cs0:cs0 + nch, :], tbl,
                        idx_sb[:, col0 + cs0 * 8: col0 + (cs0 + nch) * 8],
                        nch * 128, nch * 128, 128,
                        single_packet=GATHER_SINGLE_PACKET)
                    cs0 += nch

            for i in range(G):
                t = g * G + i
                agg = agg_ps.tile([cfg.hid, 128], f32)
                slots = ([i * cfg.k_lo + j for j in range(cfg.k_lo)]
                         + [GKLO + i * cfg.k_hi + j for j in range(cfg.k_hi)])
                for jj, cs in enumerate(slots):
                    gc = g * NCHG + cs
                    cmat = c_p.tile([128, 128], bf16)
                    nc.vector.tensor_scalar(
                        cmat[:], iota_ps[:],
                        dstloc_sb[:, gc:gc + 1],
                        dinvdst_sb[:, gc:gc + 1],
                        mybir.AluOpType.is_equal,
                        mybir.AluOpType.mult)
                    nc.tensor.matmul(
                        agg[:], msgs[:, cs, 0:cfg.hid], cmat[:],
                        start=(jj == 0), stop=(jj == len(slots) - 1))
                relu = relu_p.tile([cfg.hid, 128], bf16)
                nc.scalar.activation(
                    relu[:], agg[:], mybir.ActivationFunctionType.Relu,
                    bias=b1_sb[:])
                o2 = o2_ps.tile([cfg.out_dim, 128], f32)
                nc.tensor.matmul(o2[:], wfcT_sb[:], relu[:],
                                 start=True, stop=True)
                if i == 0:
                    ostage = ost_p.tile([cfg.out_dim, G * 128], f32)
                nc.scalar.activation(
                    ostage[:, i * 128:(i + 1) * 128], o2[:],
                    mybir.ActivationFunctionType.Identity, bias=bfc_sb[:])
            nc.sync.dma_start(
                out_d[:, g * G * 128:(g + 1) * G * 128], ostage[:])

    nc.compile()
    return nc


# ----------------------------------------------------------------------------
# Entry points.
# ----------------------------------------------------------------------------

_CACHE = {}


def _get_compiled(edges, cfg_base):
    import hashlib
    e = np.ascontiguousarray(np.asarray(edges, dtype=np.int64))
    key = (e.shape, hashlib.sha1(e.tobytes()).hexdigest(), cfg_base)
    if key not in _CACHE:
        cfg, plan = _plan(e, cfg_base)
        nc = _build_module(cfg)
        _CACHE[key] = (cfg, plan, nc)
    return _CACHE[key]


def _run(X, edges, W1, b1, Wfc, bfc, cfg_base, trace=False):
    from concourse.bass_utils import run_bass_kernel_spmd

    cfg, plan, nc = _get_compiled(edges, cfg_base)
    in_maps = _make_inputs(X, W1, b1, Wfc, bfc, cfg, plan)
    res = run_bass_kernel_spmd(
        nc, in_maps, core_ids=list(range(cfg.n_cores)), trace=trace)

    s = cfg.slots_per_core
    full = np.concatenate([res.results[c]["out"] for c in range(cfg.n_cores)],
                          axis=1)                      # [40, n_slots]
    out = full[:, plan["node_to_slot"]].T.astype(np.float32)
    out = np.ascontiguousarray(out)
    return out, res


def kernel(X, edges, W1, b1, Wfc, bfc):
    cfg_base = (N_NODES, IN_DIM, HID, OUT_DIM, N_CORES, 49, 7, 32768)
    out, _ = _run(np.asarray(X, dtype=np.float32), np.asarray(edges),
                  np.asarray(W1, dtype=np.float32),
                  np.asarray(b1, dtype=np.float32),
                  np.asarray(Wfc, dtype=np.float32),
                  np.asarray(bfc, dtype=np.float32), cfg_base)
    return out



# revision 11
# speedup vs baseline: 1.1966x; 1.1278x over previous
"""Trainium2 Bass kernel for a 2-layer GCN (GCNConv -> ReLU -> Linear).

Math (matching the PyG-style reference):
    deg  = in_degree(dst) + 1 (self loops), dinv = deg^-1/2
    h    = X @ W1                                  [N, 64]
    agg[d] = dinv[d] * sum_{e:(s->d)} dinv[s]*h[s] (+ self loop)   [N, 64]
    out  = relu(agg + b1) @ Wfc.T + bfc            [N, 40]

Distribution over 8 NeuronCores (graph/data parallel):
  - Nodes are re-labeled into 392 "tiles" of 128 slots (balanced so each
    (core, tile) has nearly equal lo/hi in-edge counts), 49 tiles per core.
    Each core computes h' = dinv*h for its 6272 slots (X @ W1 on the tensor
    engine) in two halves; each half is AllGather'd as soon as it is ready
    (two collectives, overlapped with compute / descriptor generation).
  - Each core aggregates the edges whose destination it owns: a bulk
    SWDGE dma_gather fetches h'[src] rows (256 B each) into SBUF in
    edge-major layout; destinations are scatter-added via one-hot matmuls
    (lhsT = gathered messages [128 edges, 64], rhs = C [128 edges, 128 dst]
    with C[j, d] = dinv_dst[j] * (dst_loc[j] == d)) accumulated in PSUM.
    C is built on the vector engine from a PSUM-resident iota (PSUM source
    keeps the op out of the 2-port DVE perf modes, which would otherwise
    lock GpSimd out of the shared SBUF port during SWDGE desc-gen).
  - Self loops never enter the gather stream: a per-tile diagonal matmul
    (lhsT = local h' tile, rhs = diag(dinv)) adds dinv^2*h directly.
  - relu(agg + b1) is fused into the PSUM->SBUF eviction on the scalar
    engine, the second layer is one small matmul per tile, and the bfc bias
    rides the final eviction.  The host un-permutes the [40, slots] outputs.

dma_gather indices are int16, so the table is built as TWO AllGather
outputs: half 0 = tiles 0..24 of every core (25600 rows), half 1 = tiles
25..48 (24576 rows); each tile's edges are packed into k0[t] half-0 chunks
followed by k1[t] half-1 chunks (pad lanes: idx=0, dst_loc=-1, dinv_dst=0).
The per-tile chunk-count profile (k0, k1) is shared by all cores (SPMD);
each core's tiles are assigned to profile slots by sorted edge counts.
"""

import numpy as np

# ----------------------------------------------------------------------------
# Problem configuration (hardcoded; kernel.py must be self-contained).
# ----------------------------------------------------------------------------
N_NODES = 50000
N_EDGES = 800000
IN_DIM = 512
HID = 64
OUT_DIM = 40
N_CORES = 8
TILES_PER_CORE = 49
GROUP = 7                    # tiles per gather/staging group
HALF_T = 25                  # tiles 0..24 are table half 0

GATHER_MAX_CHUNKS = 18
GATHER_SINGLE_PACKET = False


class Cfg:
    def __init__(self, n_nodes, in_dim, hid, out_dim, n_cores, k0, k1):
        self.n_nodes = n_nodes
        self.in_dim = in_dim
        self.hid = hid
        self.out_dim = out_dim
        self.n_cores = n_cores
        self.nt = TILES_PER_CORE
        self.group = GROUP
        self.n_groups = self.nt // self.group
        self.slots_per_core = self.nt * 128
        self.n_tiles = n_cores * self.nt
        self.n_slots = self.n_tiles * 128
        self.half0_rows = n_cores * HALF_T * 128       # 25600
        self.half1_rows = n_cores * (self.nt - HALF_T) * 128   # 24576
        self.k0 = k0                  # per-tile half-0 chunk counts [nt]
        self.k1 = k1                  # per-tile half-1 chunk counts [nt]
        self.n_chunks = int(np.sum(k0) + np.sum(k1))
        self.kd = in_dim // 128
        assert in_dim % 128 == 0
        assert self.half0_rows <= 32768 and self.half1_rows <= 32768


def _slot_of(tile, lane, n_cores):
    """Global table row of (global tile, lane) under the two-half layout."""
    core = tile // TILES_PER_CORE
    t = tile % TILES_PER_CORE
    half0 = n_cores * HALF_T * 128
    return np.where(
        t < HALF_T,
        core * (HALF_T * 128) + t * 128 + lane,
        half0 + core * ((TILES_PER_CORE - HALF_T) * 128) + (t - HALF_T) * 128
        + lane,
    )


# ----------------------------------------------------------------------------
# Host-side graph preprocessing (index/layout work only; all feature math
# runs on the device).
# ----------------------------------------------------------------------------

def _plan(edges):
    n_nodes, n_cores = N_NODES, N_CORES
    nt = TILES_PER_CORE
    n_tiles = n_cores * nt

    src = np.asarray(edges[0], dtype=np.int64)
    dst = np.asarray(edges[1], dtype=np.int64)
    deg = np.bincount(dst, minlength=n_nodes).astype(np.int64) + 1
    dinv = (1.0 / np.sqrt(deg.astype(np.float64))).astype(np.float32)

    # Snake binpack nodes (by degree, desc) into n_tiles bins of <=128 slots.
    order = np.argsort(-deg, kind="stable")
    rounds = np.arange(n_nodes) // n_tiles
    pos = np.arange(n_nodes) % n_tiles
    tile_of = np.where(rounds % 2 == 0, pos, n_tiles - 1 - pos)
    assert rounds.max() < 128, "more than 128 slots per tile"
    node_tile = np.empty(n_nodes, dtype=np.int64)
    node_lane = np.empty(n_nodes, dtype=np.int64)
    node_tile[order] = tile_of
    node_lane[order] = rounds

    half0 = n_cores * HALF_T * 128
    d_tile = node_tile[dst]
    d_lane = node_lane[dst]
    d_core = d_tile // nt
    d_tloc = d_tile % nt

    # The AllGather table stores each core's tiles in PROFILE-SLOT order
    # (stage column p holds local tile sort_t[c, p]), so an edge's class
    # (which table half its source lives in) depends on the source tile's
    # profile slot — which depends on per-class counts.  Two rounds: round 1
    # assigns profile slots using real-tile classes; round 2 recomputes
    # classes/counts under that fixed assignment.
    s_core = node_tile[src] // nt
    s_tloc = node_tile[src] % nt
    prof_slot = np.tile(np.arange(nt), (n_cores, 1))
    sort_t = np.tile(np.arange(nt), (n_cores, 1))
    for _round in range(2):
        s_p = prof_slot[s_core, s_tloc]
        cls = (s_p >= HALF_T).astype(np.int64)
        cnt = np.zeros((n_cores, nt, 2), dtype=np.int64)
        np.add.at(cnt, (d_core, d_tloc, cls), 1)
        key = cnt[:, :, 0] * 100000 + cnt[:, :, 1]
        sort_t = np.argsort(-key, axis=1, kind="stable")   # [cores, nt]
        prof_slot = np.empty((n_cores, nt), dtype=np.int64)
        for c in range(n_cores):
            prof_slot[c, sort_t[c]] = np.arange(nt)

    s_p = prof_slot[s_core, s_tloc]
    cls = (s_p >= HALF_T).astype(np.int64)
    cnt = np.zeros((n_cores, nt, 2), dtype=np.int64)
    np.add.at(cnt, (d_core, d_tloc, cls), 1)
    cnt_sorted = np.take_along_axis(cnt, sort_t[:, :, None], axis=1)
    k0 = np.max((cnt_sorted[:, :, 0] + 127) // 128, axis=0).astype(np.int64)
    k1 = np.max((cnt_sorted[:, :, 1] + 127) // 128, axis=0).astype(np.int64)
    k0 = np.maximum(k0, 1)
    k1 = np.maximum(k1, 1)
    # profile slots 0..HALF_T-1 must be the half-0 chunk-heavy ones?  No —
    # halves are POSITIONS: table half 0 = profile slots < HALF_T of every
    # core.  (sort order only balances counts.)

    cfg = Cfg(n_nodes, IN_DIM, HID, OUT_DIM, n_cores, k0, k1)

    # Source table row of each edge (profile-slot based).
    s_row = _slot_of(s_core * nt + s_p, node_lane[src], n_cores)

    slot_dinv = np.zeros(n_cores * nt * 128, dtype=np.float32)
    slot_dinv[_slot_of(node_tile, node_lane, n_cores)] = dinv

    # Chunk-slot numbering (shared across cores): profile slots are laid out
    # group-major; within a group: all half-0 chunks (tile-major), then all
    # half-1 chunks.
    #   base0[g] = start of group g's chunk range
    c0_in_g = np.zeros(nt, dtype=np.int64)   # chunk offset of tile within grp
    c1_in_g = np.zeros(nt, dtype=np.int64)
    g_nch = np.zeros(cfg.n_groups, dtype=np.int64)
    g_nch0 = np.zeros(cfg.n_groups, dtype=np.int64)
    g_base = np.zeros(cfg.n_groups, dtype=np.int64)
    acc = 0
    for g in range(cfg.n_groups):
        ts = np.arange(g * GROUP, (g + 1) * GROUP)
        g_base[g] = acc
        off = 0
        for t in ts:
            c0_in_g[t] = off
            off += k0[t]
        g_nch0[g] = off
        for t in ts:
            c1_in_g[t] = off
            off += k1[t]
        g_nch[g] = off
        acc += off
    n_chunks = acc
    assert n_chunks == cfg.n_chunks

    # Rank edges within (core, profile slot, class).
    p_slot = prof_slot[d_core, d_tloc]
    ekey = ((d_core * nt + p_slot) * 2 + cls)
    sort_idx = np.argsort(ekey, kind="stable")
    ekey_s = ekey[sort_idx]
    starts = np.searchsorted(ekey_s, np.arange(n_cores * nt * 2))
    rank = np.arange(len(ekey_s)) - starts[ekey_s]

    es_core = d_core[sort_idx]
    es_pslot = p_slot[sort_idx]
    es_cls = cls[sort_idx]
    es_g = es_pslot // GROUP
    j_chunk = rank >> 7
    lane = rank & 127
    fc = np.where(
        es_cls == 0,
        g_base[es_g] + c0_in_g[es_pslot] + j_chunk,
        g_base[es_g] + c1_in_g[es_pslot] + j_chunk,
    )
    assert (j_chunk < np.where(es_cls == 0, k0[es_pslot], k1[es_pslot])).all()

    idx16 = np.zeros((n_cores, n_chunks, 128), dtype=np.int16)
    dstloc = np.full((n_cores, n_chunks, 128), -1.0, dtype=np.float32)
    dinvdst = np.zeros((n_cores, n_chunks, 128), dtype=np.float32)

    s_rel = np.where(es_cls == 0, s_row[sort_idx],
                     s_row[sort_idx] - half0).astype(np.int16)
    idx16[es_core, fc, lane] = s_rel
    dstloc[es_core, fc, lane] = d_lane[sort_idx].astype(np.float32)
    dinvdst[es_core, fc, lane] = dinv[dst[sort_idx]]

    # Wrap gather indices: per (group, class) region, list position s ->
    # partition s%16, column s//16; replicated across the 8 q7 cores.
    n_idx_cols = n_chunks * 8
    idx_wrapped = np.zeros((n_cores, 128, n_idx_cols), dtype=np.int16)
    for g in range(cfg.n_groups):
        for lo, hi in ((0, g_nch0[g]), (g_nch0[g], g_nch[g])):
            nch = hi - lo
            if nch == 0:
                continue
            fc0 = g_base[g] + lo
            flat = idx16[:, fc0:fc0 + nch, :].reshape(n_cores, nch * 128)
            wrapped = flat.reshape(n_cores, nch * 8, 16).transpose(0, 2, 1)
            c0 = fc0 * 8
            idx_wrapped[:, :16, c0:c0 + nch * 8] = wrapped
    idx_wrapped[:, 16:, :] = np.tile(idx_wrapped[:, :16, :], (1, 7, 1))

    # Per-profile-slot dinv columns (for h' scaling and diag matmuls): the
    # lanes of (core, profile slot p) are those of its assigned local tile.
    # Phase-1 stage is laid out in PROFILE-SLOT order so that gather chunk /
    # matmul structure is SPMD-uniform; the AllGather table rows follow the
    # same order, and node_to_slot already accounts for it via _slot_of on
    # REAL tiles... stage column p holds local tile sort_t[c, p].
    stage_tile = sort_t                                   # [cores, nt]
    # dinv per (core, profile slot, lane):
    dinv_ps = np.zeros((n_cores, 128, nt), dtype=np.float32)
    for c in range(n_cores):
        for p in range(nt):
            t = stage_tile[c, p]
            gt = c * nt + t
            sl = _slot_of(np.full(128, gt), np.arange(128), n_cores)
            dinv_ps[c, :, p] = slot_dinv[sl]

    plan = dict(
        node_tile=node_tile, node_lane=node_lane,
        slot_dinv=slot_dinv, stage_tile=stage_tile,
        idx_wrapped=idx_wrapped,
        dstloc=dstloc.transpose(0, 2, 1).copy(),
        dinvdst=dinvdst.transpose(0, 2, 1).copy(),
        dinv_ps=dinv_ps,
        g_nch=g_nch, g_nch0=g_nch0, g_base=g_base,
        c0_in_g=c0_in_g, c1_in_g=c1_in_g,
    )
    return cfg, plan


def _make_inputs(X, W1, b1, Wfc, bfc, cfg, plan):
    """Build the 8 per-core input dicts for run_bass_kernel_spmd."""
    import ml_dtypes
    bf16 = ml_dtypes.bfloat16
    nt = cfg.nt

    # X rows arranged per (core, PROFILE slot, lane): stage column p of core
    # c holds local tile stage_tile[c, p].
    Xp = np.zeros((cfg.n_slots, cfg.in_dim), dtype=np.float32)
    # destination row for node n: core*nt*128 + prof_slot... we need X in the
    # ORDER phase 1 consumes it: xt[:, k, p*128+lane] = X[node at (c,p,lane)]
    node_tile = plan["node_tile"]
    node_lane = plan["node_lane"]
    stage_tile = plan["stage_tile"]
    inv_stage = np.empty_like(stage_tile)
    for c in range(cfg.n_cores):
        inv_stage[c, stage_tile[c]] = np.arange(nt)
    n_core = node_tile // nt
    n_tloc = node_tile % nt
    n_p = inv_stage[n_core, n_tloc]
    xrow = n_core * (nt * 128) + n_p * 128 + node_lane
    Xp[xrow] = np.asarray(X, dtype=np.float32)

    W1r = (np.asarray(W1, dtype=np.float32)
           .reshape(cfg.kd, 128, cfg.hid).transpose(1, 0, 2)
           .reshape(128, cfg.kd * cfg.hid).astype(bf16))
    wfcT = np.ascontiguousarray(np.asarray(Wfc, dtype=np.float32).T).astype(bf16)
    b1c = np.asarray(b1, dtype=np.float32).reshape(cfg.hid, 1)
    bfcc = np.asarray(bfc, dtype=np.float32).reshape(cfg.out_dim, 1)
    iota = np.ascontiguousarray(
        np.tile(np.arange(128, dtype=np.float32), (128, 1)))
    partcol = np.arange(128, dtype=np.float32).reshape(128, 1)

    s = cfg.slots_per_core
    in_maps = []
    for c in range(cfg.n_cores):
        xt = np.ascontiguousarray(Xp[c * s:(c + 1) * s].T).astype(bf16)
        in_maps.append({
            "xt": xt,
            "w1": W1r,
            "wfcT": wfcT,
            "b1": b1c,
            "bfc": bfcc,
            "iota": iota,
            "partcol": partcol,
            "dinv_sb": np.ascontiguousarray(plan["dinv_ps"][c]),
            "idx": plan["idx_wrapped"][c],
            "dstloc": plan["dstloc"][c],
            "dinvdst": plan["dinvdst"][c],
        })
    return in_maps


# ----------------------------------------------------------------------------
# Device kernel.
# ----------------------------------------------------------------------------

def _build_module(cfg, plan):
    import concourse.bass as bass
    import concourse.bacc as bacc
    import concourse.mybir as mybir
    import concourse.tile as tile
    from contextlib import ExitStack

    f32 = mybir.dt.float32
    bf16 = mybir.dt.bfloat16
    i16 = mybir.dt.int16
    S = cfg.slots_per_core
    G = cfg.group
    nt = cfg.nt
    n_chunks = cfg.n_chunks
    n_idx_cols = n_chunks * 8
    g_nch = plan["g_nch"]
    g_nch0 = plan["g_nch0"]
    g_base = plan["g_base"]
    c0_in_g = plan["c0_in_g"]
    c1_in_g = plan["c1_in_g"]
    k0, k1 = cfg.k0, cfg.k1
    H0T = HALF_T                 # tiles in half 0
    H1T = nt - HALF_T
    NCHG_MAX = int(np.max(g_nch))

    nc = bacc.Bacc("TRN2", target_bir_lowering=False, debug=False,
                   num_devices=cfg.n_cores)

    xt_d = nc.dram_tensor("xt", [cfg.in_dim, S], bf16, kind="ExternalInput")
    w1_d = nc.dram_tensor("w1", [128, cfg.kd * cfg.hid], bf16,
                          kind="ExternalInput")
    wfcT_d = nc.dram_tensor("wfcT", [cfg.hid, cfg.out_dim], bf16,
                            kind="ExternalInput")
    b1_d = nc.dram_tensor("b1", [cfg.hid, 1], f32, kind="ExternalInput")
    bfc_d = nc.dram_tensor("bfc", [cfg.out_dim, 1], f32, kind="ExternalInput")
    iota_d = nc.dram_tensor("iota", [128, 128], f32, kind="ExternalInput")
    partcol_d = nc.dram_tensor("partcol", [128, 1], f32, kind="ExternalInput")
    dinv_d = nc.dram_tensor("dinv_sb", [128, nt], f32, kind="ExternalInput")
    idx_d = nc.dram_tensor("idx", [128, n_idx_cols], i16, kind="ExternalInput")
    dstloc_d = nc.dram_tensor("dstloc", [128, n_chunks], f32,
                              kind="ExternalInput")
    dinvdst_d = nc.dram_tensor("dinvdst", [128, n_chunks], f32,
                               kind="ExternalInput")
    out_d = nc.dram_tensor("out", [cfg.out_dim, S], f32, kind="ExternalOutput")

    with tile.TileContext(nc) as tc, ExitStack() as ctx:
        dram = ctx.enter_context(tc.tile_pool(name="dram", bufs=1,
                                              space="DRAM"))
        consts = ctx.enter_context(tc.tile_pool(name="consts", bufs=1))
        persist = ctx.enter_context(tc.tile_pool(name="persist", bufs=1))
        ag0_in = dram.tile([H0T * 128, 128], bf16)
        ag1_in = dram.tile([H1T * 128, 128], bf16)
        ag0_out = dram.tile([cfg.half0_rows, 128], bf16, addr_space="Shared")
        ag1_out = dram.tile([cfg.half1_rows, 128], bf16, addr_space="Shared")

        # iota lives in PSUM: a PSUM-source tensor_scalar cannot enter the
        # 2-port DVE perf modes, so the per-chunk C-matrix builds never grab
        # the shared SBUF port pair that SWDGE descriptor generation (the
        # dma_gather Q7 loop) needs — they'd otherwise serialize.
        iota_pp = ctx.enter_context(
            tc.tile_pool(name="iotapp", bufs=1, space="PSUM"))
        iota_ps = iota_pp.tile([128, 128], f32)

        iota_sb = consts.tile([128, 128], f32)
        partcol_sb = consts.tile([128, 1], f32)
        w1_sb = consts.tile([128, cfg.kd * cfg.hid], bf16)
        wfcT_sb = consts.tile([cfg.hid, cfg.out_dim], bf16)
        b1_sb = consts.tile([cfg.hid, 1], f32)
        bfc_sb = consts.tile([cfg.out_dim, 1], f32)
        dinv_sb = consts.tile([128, nt], f32)
        idx_sb = consts.tile([128, n_idx_cols], i16)
        dstloc_sb = consts.tile([128, n_chunks], f32)
        dinvdst_sb = consts.tile([128, n_chunks], f32)
        stage = persist.tile([128, nt, 128], bf16)
        xt_sb = persist.tile([128, cfg.kd, S], bf16)

        # xt first (phase 1's critical input), then the rest.
        nc.sync.dma_start(
            xt_sb[:, :, 0:H0T * 128],
            xt_d[:, 0:H0T * 128].rearrange("(k p) s -> p k s", p=128))
        nc.sync.dma_start(
            xt_sb[:, :, H0T * 128:S],
            xt_d[:, H0T * 128:S].rearrange("(k p) s -> p k s", p=128))
        nc.sync.dma_start(iota_sb[:], iota_d[:])
        nc.sync.dma_start(partcol_sb[:], partcol_d[:])
        nc.sync.dma_start(w1_sb[:], w1_d[:])
        nc.sync.dma_start(wfcT_sb[:], wfcT_d[:])
        nc.sync.dma_start(b1_sb[:], b1_d[:])
        nc.sync.dma_start(bfc_sb[:], bfc_d[:])
        nc.sync.dma_start(dinv_sb[:], dinv_d[:])
        nc.sync.dma_start(idx_sb[:], idx_d[:])
        nc.sync.dma_start(dstloc_sb[:], dstloc_d[:])
        nc.sync.dma_start(dinvdst_sb[:], dinvdst_d[:])
        nc.vector.tensor_scalar_mul(iota_ps[:], iota_sb[:], 1.0)

        # ---- Phase 1: h' = dinv * (X @ W1), bf16 rows padded to 256 B ----
        # Computed in two halves; each half is AllGather'd as soon as ready.
        with tc.tile_pool(name="p1ps", bufs=2, space="PSUM") as p1ps:
            for half, (t0, t1, ag_in, ag_out, rows) in enumerate((
                    (0, H0T, ag0_in, ag0_out, cfg.half0_rows),
                    (H0T, nt, ag1_in, ag1_out, cfg.half1_rows))):
                for t in range(t0, t1):
                    ph = p1ps.tile([128, cfg.hid], f32)
                    for k in range(cfg.kd):
                        nc.tensor.matmul(
                            ph[:],
                            xt_sb[:, k, t * 128:(t + 1) * 128],
                            w1_sb[:, k * cfg.hid:(k + 1) * cfg.hid],
                            start=(k == 0), stop=(k == cfg.kd - 1))
                    nc.vector.tensor_scalar_mul(
                        stage[:, t, 0:cfg.hid], ph[:],
                        dinv_sb[:, t:t + 1])
                nc.sync.dma_start(
                    ag_in[:].rearrange("(t p) e -> p t e", p=128),
                    stage[:, t0:t1, :])
                nc.gpsimd.collective_compute(
                    "AllGather",
                    mybir.AluOpType.bypass,
                    ins=[ag_in.opt()],
                    outs=[ag_out.opt()],
                    replica_groups=[list(range(cfg.n_cores))],
                )

        # ---- Phase 2: gather + one-hot scatter matmuls + layer 2 ----
        msgs_p = ctx.enter_context(tc.tile_pool(name="msgs", bufs=2))
        c_p = ctx.enter_context(tc.tile_pool(name="cmat", bufs=8))
        relu_p = ctx.enter_context(tc.tile_pool(name="relu", bufs=3))
        ost_p = ctx.enter_context(tc.tile_pool(name="ost", bufs=2))
        agg_ps = ctx.enter_context(
            tc.tile_pool(name="aggps", bufs=4, space="PSUM"))
        o2_ps = ctx.enter_context(
            tc.tile_pool(name="o2ps", bufs=2, space="PSUM"))

        GMAX = GATHER_MAX_CHUNKS

        for g in range(cfg.n_groups):
            nchg = int(g_nch[g])
            nch0 = int(g_nch0[g])
            base = int(g_base[g])
            msgs = msgs_p.tile([128, NCHG_MAX, 128], bf16)
            col0 = base * 8
            for r0, r1, tbl in ((0, nch0, ag0_out[:, :]),
                                (nch0, nchg, ag1_out[:, :])):
                cs0 = r0
                while cs0 < r1:
                    nch = min(GMAX, r1 - cs0)
                    nc.gpsimd.dma_gather(
                        msgs[:, cs0:cs0 + nch, :], tbl,
                        idx_sb[:, col0 + cs0 * 8: col0 + (cs0 + nch) * 8],
                        nch * 128, nch * 128, 128,
                        single_packet=GATHER_SINGLE_PACKET)
                    cs0 += nch

            for i in range(G):
                p = g * G + i
                agg = agg_ps.tile([cfg.hid, 128], f32)
                slots = ([int(c0_in_g[p]) + j for j in range(int(k0[p]))]
                         + [int(c1_in_g[p]) + j for j in range(int(k1[p]))])
                for cs in slots:
                    gc = base + cs
                    cmat = c_p.tile([128, 128], bf16)
                    nc.vector.tensor_scalar(
                        cmat[:], iota_ps[:],
                        dstloc_sb[:, gc:gc + 1],
                        dinvdst_sb[:, gc:gc + 1],
                        mybir.AluOpType.is_equal,
                        mybir.AluOpType.mult)
                    nc.tensor.matmul(
                        agg[:], msgs[:, cs, 0:cfg.hid], cmat[:],
                        start=(cs == slots[0]), stop=False)
                # self-loop term: lhsT = stage tile (dinv*h), rhs = diag(dinv)
                dmat = c_p.tile([128, 128], bf16)
                nc.vector.tensor_scalar(
                    dmat[:], iota_ps[:],
                    partcol_sb[:, 0:1],
                    dinv_sb[:, p:p + 1],
                    mybir.AluOpType.is_equal,
                    mybir.AluOpType.mult)
                nc.tensor.matmul(
                    agg[:], stage[:, p, 0:cfg.hid], dmat[:],
                    start=False, stop=True)
                relu = relu_p.tile([cfg.hid, 128], bf16)
                nc.scalar.activation(
                    relu[:], agg[:], mybir.ActivationFunctionType.Relu,
                    bias=b1_sb[:])
                o2 = o2_ps.tile([cfg.out_dim, 128], f32)
                nc.tensor.matmul(o2[:], wfcT_sb[:], relu[:],
                                 start=True, stop=True)
                if i == 0:
                    ostage = ost_p.tile([cfg.out_dim, G * 128], f32)
                nc.scalar.activation(
                    ostage[:, i * 128:(i + 1) * 128], o2[:],
                    mybir.ActivationFunctionType.Identity, bias=bfc_sb[:])
            nc.sync.dma_start(
                out_d[:, g * G * 128:(g + 1) * G * 128], ostage[:])

    nc.compile()
    return nc


# ----------------------------------------------------------------------------
# Entry points.
# ----------------------------------------------------------------------------

_CACHE = {}


def _get_compiled(edges):
    import hashlib
    e = np.ascontiguousarray(np.asarray(edges, dtype=np.int64))
    key = (e.shape, hashlib.sha1(e.tobytes()).hexdigest())
    if key not in _CACHE:
        cfg, plan = _plan(e)
        nc = _build_module(cfg, plan)
        _CACHE[key] = (cfg, plan, nc)
    return _CACHE[key]


def _run(X, edges, W1, b1, Wfc, bfc, trace=False):
    from concourse.bass_utils import run_bass_kernel_spmd

    cfg, plan, nc = _get_compiled(edges)
    in_maps = _make_inputs(X, W1, b1, Wfc, bfc, cfg, plan)
    res = run_bass_kernel_spmd(
        nc, in_maps, core_ids=list(range(cfg.n_cores)), trace=trace)

    # Device output column (c, p, lane) -> node via stage_tile mapping.
    nt = cfg.nt
    full = np.concatenate([res.results[c]["out"] for c in range(cfg.n_cores)],
                          axis=1)                      # [40, n_slots]
    node_tile = plan["node_tile"]
    node_lane = plan["node_lane"]
    stage_tile = plan["stage_tile"]
    inv_stage = np.empty_like(stage_tile)
    for c in range(cfg.n_cores):
        inv_stage[c, stage_tile[c]] = np.arange(nt)
    n_core = node_tile // nt
    n_p = inv_stage[n_core, node_tile % nt]
    col = n_core * (nt * 128) + n_p * 128 + node_lane
    out = full[:, col].T.astype(np.float32)
    out = np.ascontiguousarray(out)
    return out, res


def kernel(X, edges, W1, b1, Wfc, bfc):
    out, _ = _run(np.asarray(X, dtype=np.float32), np.asarray(edges),
                  np.asarray(W1, dtype=np.float32),
                  np.asarray(b1, dtype=np.float32),
                  np.asarray(Wfc, dtype=np.float32),
                  np.asarray(bfc, dtype=np.float32))
    return out


# revision 13
# speedup vs baseline: 1.7395x; 1.4537x over previous
"""Trainium2 Bass kernel for a 2-layer GCN (GCNConv -> ReLU -> Linear).

Math (matching the PyG-style reference):
    deg  = in_degree(dst) + 1 (self loops), dinv = deg^-1/2
    h    = X @ W1                                  [N, 64]
    agg[d] = dinv[d] * sum_{e:(s->d)} dinv[s]*h[s] (+ self loop)   [N, 64]
    out  = relu(agg + b1) @ Wfc.T + bfc            [N, 40]

Distribution over 8 NeuronCores (graph/data parallel):
  - Nodes are re-labeled into 392 "tiles" of 128 slots (balanced so each
    (core, tile) has nearly equal lo/hi in-edge counts), 49 tiles per core.
    Each core computes h' = dinv*h for its 6272 slots (X @ W1 on the tensor
    engine) in two halves; each half is AllGather'd as soon as it is ready
    (two collectives, overlapped with compute / descriptor generation).
  - Each core aggregates the edges whose destination it owns: a bulk
    SWDGE dma_gather fetches h'[src] rows (256 B each) into SBUF in
    edge-major layout; destinations are scatter-added via one-hot matmuls
    (lhsT = gathered messages [128 edges, 64], rhs = C [128 edges, 128 dst]
    with C[j, d] = dinv_dst[j] * (dst_loc[j] == d)) accumulated in PSUM.
    C is built on the vector engine from a PSUM-resident iota (PSUM source
    keeps the op out of the 2-port DVE perf modes, which would otherwise
    lock GpSimd out of the shared SBUF port during SWDGE desc-gen).
  - Self loops never enter the gather stream: a per-tile diagonal matmul
    (lhsT = local h' tile, rhs = diag(dinv)) adds dinv^2*h directly.
  - relu(agg + b1) is fused into the PSUM->SBUF eviction on the scalar
    engine, the second layer is one small matmul per tile, and the bfc bias
    rides the final eviction.  The host un-permutes the [40, slots] outputs.

dma_gather indices are int16, so the table is built as TWO AllGather
outputs: half 0 = tiles 0..24 of every core (25600 rows), half 1 = tiles
25..48 (24576 rows); each tile's edges are packed into k0[t] half-0 chunks
followed by k1[t] half-1 chunks (pad lanes: idx=0, dst_loc=-1, dinv_dst=0).
The per-tile chunk-count profile (k0, k1) is shared by all cores (SPMD);
each core's tiles are assigned to profile slots by sorted edge counts.
"""

import numpy as np

# ----------------------------------------------------------------------------
# Problem configuration (hardcoded; kernel.py must be self-contained).
# ----------------------------------------------------------------------------
N_NODES = 50000
N_EDGES = 800000
IN_DIM = 512
HID = 64
OUT_DIM = 40
N_CORES = 8
TILES_PER_CORE = 49
GROUP = 7                    # tiles per gather/staging group
HALF_T = 25                  # tiles 0..24 are table half 0

GATHER_MAX_CHUNKS = 18
GATHER_SINGLE_PACKET = False


class Cfg:
    def __init__(self, n_nodes, in_dim, hid, out_dim, n_cores, k0, k1):
        self.n_nodes = n_nodes
        self.in_dim = in_dim
        self.hid = hid
        self.out_dim = out_dim
        self.n_cores = n_cores
        self.nt = TILES_PER_CORE
        self.group = GROUP
        self.n_groups = self.nt // self.group
        self.slots_per_core = self.nt * 128
        self.n_tiles = n_cores * self.nt
        self.n_slots = self.n_tiles * 128
        self.half0_rows = n_cores * HALF_T * 128       # 25600
        self.half1_rows = n_cores * (self.nt - HALF_T) * 128   # 24576
        self.k0 = k0                  # per-tile half-0 chunk counts [nt]
        self.k1 = k1                  # per-tile half-1 chunk counts [nt]
        self.n_chunks = int(np.sum(k0) + np.sum(k1))
        self.kd = in_dim // 128
        assert in_dim % 128 == 0
        assert self.half0_rows <= 32768 and self.half1_rows <= 32768


def _slot_of(tile, lane, n_cores):
    """Global table row of (global tile, lane) under the two-half layout."""
    core = tile // TILES_PER_CORE
    t = tile % TILES_PER_CORE
    half0 = n_cores * HALF_T * 128
    return np.where(
        t < HALF_T,
        core * (HALF_T * 128) + t * 128 + lane,
        half0 + core * ((TILES_PER_CORE - HALF_T) * 128) + (t - HALF_T) * 128
        + lane,
    )


# ----------------------------------------------------------------------------
# Host-side graph preprocessing (index/layout work only; all feature math
# runs on the device).
# ----------------------------------------------------------------------------

def _plan(edges):
    n_nodes, n_cores = N_NODES, N_CORES
    nt = TILES_PER_CORE
    n_tiles = n_cores * nt

    src = np.asarray(edges[0], dtype=np.int64)
    dst = np.asarray(edges[1], dtype=np.int64)
    deg = np.bincount(dst, minlength=n_nodes).astype(np.int64) + 1
    dinv = (1.0 / np.sqrt(deg.astype(np.float64))).astype(np.float32)

    # Snake binpack nodes (by degree, desc) into n_tiles bins of <=128 slots.
    order = np.argsort(-deg, kind="stable")
    rounds = np.arange(n_nodes) // n_tiles
    pos = np.arange(n_nodes) % n_tiles
    tile_of = np.where(rounds % 2 == 0, pos, n_tiles - 1 - pos)
    assert rounds.max() < 128, "more than 128 slots per tile"
    node_tile = np.empty(n_nodes, dtype=np.int64)
    node_lane = np.empty(n_nodes, dtype=np.int64)
    node_tile[order] = tile_of
    node_lane[order] = rounds

    half0 = n_cores * HALF_T * 128
    d_tile = node_tile[dst]
    d_lane = node_lane[dst]
    d_core = d_tile // nt
    d_tloc = d_tile % nt

    # The AllGather table stores each core's tiles in PROFILE-SLOT order
    # (stage column p holds local tile sort_t[c, p]), so an edge's class
    # (which table half its source lives in) depends on the source tile's
    # profile slot — which depends on per-class counts.  Two rounds: round 1
    # assigns profile slots using real-tile classes; round 2 recomputes
    # classes/counts under that fixed assignment.
    s_core = node_tile[src] // nt
    s_tloc = node_tile[src] % nt
    prof_slot = np.tile(np.arange(nt), (n_cores, 1))
    sort_t = np.tile(np.arange(nt), (n_cores, 1))
    for _round in range(2):
        s_p = prof_slot[s_core, s_tloc]
        cls = (s_p >= HALF_T).astype(np.int64)
        cnt = np.zeros((n_cores, nt, 2), dtype=np.int64)
        np.add.at(cnt, (d_core, d_tloc, cls), 1)
        key = cnt[:, :, 0] * 100000 + cnt[:, :, 1]
        sort_t = np.argsort(-key, axis=1, kind="stable")   # [cores, nt]
        prof_slot = np.empty((n_cores, nt), dtype=np.int64)
        for c in range(n_cores):
            prof_slot[c, sort_t[c]] = np.arange(nt)

    s_p = prof_slot[s_core, s_tloc]
    cls = (s_p >= HALF_T).astype(np.int64)
    cnt = np.zeros((n_cores, nt, 2), dtype=np.int64)
    np.add.at(cnt, (d_core, d_tloc, cls), 1)
    cnt_sorted = np.take_along_axis(cnt, sort_t[:, :, None], axis=1)
    k0 = np.max((cnt_sorted[:, :, 0] + 127) // 128, axis=0).astype(np.int64)
    k1 = np.max((cnt_sorted[:, :, 1] + 127) // 128, axis=0).astype(np.int64)
    k0 = np.maximum(k0, 1)
    k1 = np.maximum(k1, 1)
    # profile slots 0..HALF_T-1 must be the half-0 chunk-heavy ones?  No —
    # halves are POSITIONS: table half 0 = profile slots < HALF_T of every
    # core.  (sort order only balances counts.)

    cfg = Cfg(n_nodes, IN_DIM, HID, OUT_DIM, n_cores, k0, k1)

    # Source table row of each edge (profile-slot based).
    s_row = _slot_of(s_core * nt + s_p, node_lane[src], n_cores)

    slot_dinv = np.zeros(n_cores * nt * 128, dtype=np.float32)
    slot_dinv[_slot_of(node_tile, node_lane, n_cores)] = dinv

    # Chunk-slot numbering (shared across cores): profile slots are laid out
    # group-major; within a group: all half-0 chunks (tile-major), then all
    # half-1 chunks.
    #   base0[g] = start of group g's chunk range
    c0_in_g = np.zeros(nt, dtype=np.int64)   # chunk offset of tile within grp
    c1_in_g = np.zeros(nt, dtype=np.int64)
    g_nch = np.zeros(cfg.n_groups, dtype=np.int64)
    g_nch0 = np.zeros(cfg.n_groups, dtype=np.int64)
    g_base = np.zeros(cfg.n_groups, dtype=np.int64)
    acc = 0
    for g in range(cfg.n_groups):
        ts = np.arange(g * GROUP, (g + 1) * GROUP)
        g_base[g] = acc
        off = 0
        for t in ts:
            c0_in_g[t] = off
            off += k0[t]
        g_nch0[g] = off
        for t in ts:
            c1_in_g[t] = off
            off += k1[t]
        g_nch[g] = off
        acc += off
    n_chunks = acc
    assert n_chunks == cfg.n_chunks

    # Rank edges within (core, profile slot, class).
    p_slot = prof_slot[d_core, d_tloc]
    ekey = ((d_core * nt + p_slot) * 2 + cls)
    sort_idx = np.argsort(ekey, kind="stable")
    ekey_s = ekey[sort_idx]
    starts = np.searchsorted(ekey_s, np.arange(n_cores * nt * 2))
    rank = np.arange(len(ekey_s)) - starts[ekey_s]

    es_core = d_core[sort_idx]
    es_pslot = p_slot[sort_idx]
    es_cls = cls[sort_idx]
    es_g = es_pslot // GROUP
    j_chunk = rank >> 7
    lane = rank & 127
    fc = np.where(
        es_cls == 0,
        g_base[es_g] + c0_in_g[es_pslot] + j_chunk,
        g_base[es_g] + c1_in_g[es_pslot] + j_chunk,
    )
    assert (j_chunk < np.where(es_cls == 0, k0[es_pslot], k1[es_pslot])).all()

    idx16 = np.zeros((n_cores, n_chunks, 128), dtype=np.int16)
    dstloc = np.full((n_cores, n_chunks, 128), -1.0, dtype=np.float32)
    dinvdst = np.zeros((n_cores, n_chunks, 128), dtype=np.float32)

    s_rel = np.where(es_cls == 0, s_row[sort_idx],
                     s_row[sort_idx] - half0).astype(np.int16)
    idx16[es_core, fc, lane] = s_rel
    dstloc[es_core, fc, lane] = d_lane[sort_idx].astype(np.float32)
    dinvdst[es_core, fc, lane] = dinv[dst[sort_idx]]

    # Wrap gather indices: per (group, class) region, list position s ->
    # partition s%16, column s//16; replicated across the 8 q7 cores.
    n_idx_cols = n_chunks * 8
    idx_wrapped = np.zeros((n_cores, 128, n_idx_cols), dtype=np.int16)
    for g in range(cfg.n_groups):
        for lo, hi in ((0, g_nch0[g]), (g_nch0[g], g_nch[g])):
            nch = hi - lo
            if nch == 0:
                continue
            fc0 = g_base[g] + lo
            flat = idx16[:, fc0:fc0 + nch, :].reshape(n_cores, nch * 128)
            wrapped = flat.reshape(n_cores, nch * 8, 16).transpose(0, 2, 1)
            c0 = fc0 * 8
            idx_wrapped[:, :16, c0:c0 + nch * 8] = wrapped
    idx_wrapped[:, 16:, :] = np.tile(idx_wrapped[:, :16, :], (1, 7, 1))

    # Per-profile-slot dinv columns (for h' scaling and diag matmuls): the
    # lanes of (core, profile slot p) are those of its assigned local tile.
    # Phase-1 stage is laid out in PROFILE-SLOT order so that gather chunk /
    # matmul structure is SPMD-uniform; the AllGather table rows follow the
    # same order, and node_to_slot already accounts for it via _slot_of on
    # REAL tiles... stage column p holds local tile sort_t[c, p].
    stage_tile = sort_t                                   # [cores, nt]
    # dinv per (core, profile slot, lane):
    dinv_ps = np.zeros((n_cores, 128, nt), dtype=np.float32)
    for c in range(n_cores):
        for p in range(nt):
            t = stage_tile[c, p]
            gt = c * nt + t
            sl = _slot_of(np.full(128, gt), np.arange(128), n_cores)
            dinv_ps[c, :, p] = slot_dinv[sl]

    plan = dict(
        node_tile=node_tile, node_lane=node_lane,
        slot_dinv=slot_dinv, stage_tile=stage_tile,
        idx_wrapped=idx_wrapped,
        dstloc=dstloc.transpose(0, 2, 1).copy(),
        dinvdst=dinvdst.transpose(0, 2, 1).copy(),
        dinv_ps=dinv_ps,
        g_nch=g_nch, g_nch0=g_nch0, g_base=g_base,
        c0_in_g=c0_in_g, c1_in_g=c1_in_g,
    )
    return cfg, plan


def _make_inputs(X, W1, b1, Wfc, bfc, cfg, plan):
    """Build the 8 per-core input dicts for run_bass_kernel_spmd."""
    import ml_dtypes
    bf16 = ml_dtypes.bfloat16
    nt = cfg.nt

    # X rows arranged per (core, PROFILE slot, lane): stage column p of core
    # c holds local tile stage_tile[c, p].
    Xp = np.zeros((cfg.n_slots, cfg.in_dim), dtype=np.float32)
    # destination row for node n: core*nt*128 + prof_slot... we need X in the
    # ORDER phase 1 consumes it: xt[:, k, p*128+lane] = X[node at (c,p,lane)]
    node_tile = plan["node_tile"]
    node_lane = plan["node_lane"]
    stage_tile = plan["stage_tile"]
    inv_stage = np.empty_like(stage_tile)
    for c in range(cfg.n_cores):
        inv_stage[c, stage_tile[c]] = np.arange(nt)
    n_core = node_tile // nt
    n_tloc = node_tile % nt
    n_p = inv_stage[n_core, n_tloc]
    xrow = n_core * (nt * 128) + n_p * 128 + node_lane
    Xp[xrow] = np.asarray(X, dtype=np.float32)

    W1r = (np.asarray(W1, dtype=np.float32)
           .reshape(cfg.kd, 128, cfg.hid).transpose(1, 0, 2)
           .reshape(128, cfg.kd * cfg.hid).astype(bf16))
    wfcT = np.ascontiguousarray(np.asarray(Wfc, dtype=np.float32).T).astype(bf16)
    b1c = np.asarray(b1, dtype=np.float32).reshape(cfg.hid, 1)
    bfcc = np.asarray(bfc, dtype=np.float32).reshape(cfg.out_dim, 1)
    iota = np.ascontiguousarray(
        np.tile(np.arange(128, dtype=np.float32), (128, 1)))
    partcol = np.arange(128, dtype=np.float32).reshape(128, 1)

    s = cfg.slots_per_core
    in_maps = []
    for c in range(cfg.n_cores):
        xt = np.ascontiguousarray(Xp[c * s:(c + 1) * s].T).astype(bf16)
        in_maps.append({
            "xt": xt,
            "w1": W1r,
            "wfcT": wfcT,
            "b1": b1c,
            "bfc": bfcc,
            "iota": iota,
            "partcol": partcol,
            "dinv_sb": np.ascontiguousarray(plan["dinv_ps"][c]),
            "idx": plan["idx_wrapped"][c],
            "dstloc": plan["dstloc"][c],
            "dinvdst": plan["dinvdst"][c],
        })
    return in_maps


# ----------------------------------------------------------------------------
# Device kernel.
# ----------------------------------------------------------------------------

def _build_module(cfg, plan):
    import concourse.bass as bass
    import concourse.bacc as bacc
    import concourse.mybir as mybir
    import concourse.tile as tile
    from contextlib import ExitStack

    f32 = mybir.dt.float32
    bf16 = mybir.dt.bfloat16
    i16 = mybir.dt.int16
    S = cfg.slots_per_core
    G = cfg.group
    nt = cfg.nt
    n_chunks = cfg.n_chunks
    n_idx_cols = n_chunks * 8
    g_nch = plan["g_nch"]
    g_nch0 = plan["g_nch0"]
    g_base = plan["g_base"]
    c0_in_g = plan["c0_in_g"]
    c1_in_g = plan["c1_in_g"]
    k0, k1 = cfg.k0, cfg.k1
    H0T = HALF_T                 # tiles in half 0
    H1T = nt - HALF_T
    NCHG_MAX = int(np.max(g_nch))

    nc = bacc.Bacc("TRN2", target_bir_lowering=False, debug=False,
                   num_devices=cfg.n_cores, num_swdge_queues=4)

    xt_d = nc.dram_tensor("xt", [cfg.in_dim, S], bf16, kind="ExternalInput")
    w1_d = nc.dram_tensor("w1", [128, cfg.kd * cfg.hid], bf16,
                          kind="ExternalInput")
    wfcT_d = nc.dram_tensor("wfcT", [cfg.hid, cfg.out_dim], bf16,
                            kind="ExternalInput")
    b1_d = nc.dram_tensor("b1", [cfg.hid, 1], f32, kind="ExternalInput")
    bfc_d = nc.dram_tensor("bfc", [cfg.out_dim, 1], f32, kind="ExternalInput")
    iota_d = nc.dram_tensor("iota", [128, 128], f32, kind="ExternalInput")
    partcol_d = nc.dram_tensor("partcol", [128, 1], f32, kind="ExternalInput")
    dinv_d = nc.dram_tensor("dinv_sb", [128, nt], f32, kind="ExternalInput")
    idx_d = nc.dram_tensor("idx", [128, n_idx_cols], i16, kind="ExternalInput")
    dstloc_d = nc.dram_tensor("dstloc", [128, n_chunks], f32,
                              kind="ExternalInput")
    dinvdst_d = nc.dram_tensor("dinvdst", [128, n_chunks], f32,
                               kind="ExternalInput")
    out_d = nc.dram_tensor("out", [cfg.out_dim, S], f32, kind="ExternalOutput")

    with tile.TileContext(nc) as tc, ExitStack() as ctx:
        dram = ctx.enter_context(tc.tile_pool(name="dram", bufs=1,
                                              space="DRAM"))
        consts = ctx.enter_context(tc.tile_pool(name="consts", bufs=1))
        persist = ctx.enter_context(tc.tile_pool(name="persist", bufs=1))
        ag0_in = dram.tile([H0T * 128, 128], bf16)
        ag1_in = dram.tile([H1T * 128, 128], bf16)
        ag0_out = dram.tile([cfg.half0_rows, 128], bf16, addr_space="Shared")
        ag1_out = dram.tile([cfg.half1_rows, 128], bf16, addr_space="Shared")

        # iota lives in PSUM: a PSUM-source tensor_scalar cannot enter the
        # 2-port DVE perf modes, so the per-chunk C-matrix builds never grab
        # the shared SBUF port pair that SWDGE descriptor generation (the
        # dma_gather Q7 loop) needs — they'd otherwise serialize.
        iota_pp = ctx.enter_context(
            tc.tile_pool(name="iotapp", bufs=1, space="PSUM"))
        iota_ps = iota_pp.tile([128, 128], f32)

        iota_sb = consts.tile([128, 128], f32)
        partcol_sb = consts.tile([128, 1], f32)
        w1_sb = consts.tile([128, cfg.kd * cfg.hid], bf16)
        wfcT_sb = consts.tile([cfg.hid, cfg.out_dim], bf16)
        b1_sb = consts.tile([cfg.hid, 1], f32)
        bfc_sb = consts.tile([cfg.out_dim, 1], f32)
        dinv_sb = consts.tile([128, nt], f32)
        idx_sb = consts.tile([128, n_idx_cols], i16)
        dstloc_sb = consts.tile([128, n_chunks], f32)
        dinvdst_sb = consts.tile([128, n_chunks], f32)
        stage = persist.tile([128, nt, 128], bf16)
        xt_sb = persist.tile([128, cfg.kd, S], bf16)

        # xt first (phase 1's critical input), then the rest.
        nc.sync.dma_start(
            xt_sb[:, :, 0:H0T * 128],
            xt_d[:, 0:H0T * 128].rearrange("(k p) s -> p k s", p=128))
        nc.sync.dma_start(
            xt_sb[:, :, H0T * 128:S],
            xt_d[:, H0T * 128:S].rearrange("(k p) s -> p k s", p=128))
        nc.sync.dma_start(iota_sb[:], iota_d[:])
        nc.sync.dma_start(partcol_sb[:], partcol_d[:])
        nc.sync.dma_start(w1_sb[:], w1_d[:])
        nc.sync.dma_start(wfcT_sb[:], wfcT_d[:])
        nc.sync.dma_start(b1_sb[:], b1_d[:])
        nc.sync.dma_start(bfc_sb[:], bfc_d[:])
        nc.sync.dma_start(dinv_sb[:], dinv_d[:])
        nc.sync.dma_start(idx_sb[:], idx_d[:])
        nc.sync.dma_start(dstloc_sb[:], dstloc_d[:])
        nc.sync.dma_start(dinvdst_sb[:], dinvdst_d[:])
        nc.vector.tensor_scalar_mul(iota_ps[:], iota_sb[:], 1.0)

        # ---- Phase 1: h' = dinv * (X @ W1), bf16 rows padded to 256 B ----
        # Computed in two halves; each half is AllGather'd as soon as ready.
        with tc.tile_pool(name="p1ps", bufs=2, space="PSUM") as p1ps:
            for half, (t0, t1, ag_in, ag_out, rows) in enumerate((
                    (0, H0T, ag0_in, ag0_out, cfg.half0_rows),
                    (H0T, nt, ag1_in, ag1_out, cfg.half1_rows))):
                for t in range(t0, t1):
                    ph = p1ps.tile([128, cfg.hid], f32)
                    for k in range(cfg.kd):
                        nc.tensor.matmul(
                            ph[:],
                            xt_sb[:, k, t * 128:(t + 1) * 128],
                            w1_sb[:, k * cfg.hid:(k + 1) * cfg.hid],
                            start=(k == 0), stop=(k == cfg.kd - 1))
                    nc.vector.tensor_scalar_mul(
                        stage[:, t, 0:cfg.hid], ph[:],
                        dinv_sb[:, t:t + 1])
                nc.sync.dma_start(
                    ag_in[:].rearrange("(t p) e -> p t e", p=128),
                    stage[:, t0:t1, :])
                nc.gpsimd.collective_compute(
                    "AllGather",
                    mybir.AluOpType.bypass,
                    ins=[ag_in.opt()],
                    outs=[ag_out.opt()],
                    replica_groups=[list(range(cfg.n_cores))],
                )

        # ---- Phase 2: gather + one-hot scatter matmuls + layer 2 ----
        msgs_p = ctx.enter_context(tc.tile_pool(name="msgs", bufs=2))
        c_p = ctx.enter_context(tc.tile_pool(name="cmat", bufs=8))
        relu_p = ctx.enter_context(tc.tile_pool(name="relu", bufs=3))
        ost_p = ctx.enter_context(tc.tile_pool(name="ost", bufs=2))
        agg_ps = ctx.enter_context(
            tc.tile_pool(name="aggps", bufs=4, space="PSUM"))
        o2_ps = ctx.enter_context(
            tc.tile_pool(name="o2ps", bufs=2, space="PSUM"))

        GMAX = GATHER_MAX_CHUNKS
        gather_call = 0

        for g in range(cfg.n_groups):
            nchg = int(g_nch[g])
            nch0 = int(g_nch0[g])
            base = int(g_base[g])
            msgs = msgs_p.tile([128, NCHG_MAX, 128], bf16)
            col0 = base * 8
            for r0, r1, tbl in ((0, nch0, ag0_out[:, :]),
                                (nch0, nchg, ag1_out[:, :])):
                cs0 = r0
                while cs0 < r1:
                    nch = min(GMAX, r1 - cs0)
                    nc.gpsimd.dma_gather(
                        msgs[:, cs0:cs0 + nch, :], tbl,
                        idx_sb[:, col0 + cs0 * 8: col0 + (cs0 + nch) * 8],
                        nch * 128, nch * 128, 128,
                        single_packet=GATHER_SINGLE_PACKET,
                        queue_num=gather_call % 4)
                    gather_call += 1
                    cs0 += nch

            for i in range(G):
                p = g * G + i
                agg = agg_ps.tile([cfg.hid, 128], f32)
                slots = ([int(c0_in_g[p]) + j for j in range(int(k0[p]))]
                         + [int(c1_in_g[p]) + j for j in range(int(k1[p]))])
                for cs in slots:
                    gc = base + cs
                    cmat = c_p.tile([128, 128], bf16)
                    nc.vector.tensor_scalar(
                        cmat[:], iota_ps[:],
                        dstloc_sb[:, gc:gc + 1],
                        dinvdst_sb[:, gc:gc + 1],
                        mybir.AluOpType.is_equal,
                        mybir.AluOpType.mult)
                    nc.tensor.matmul(
                        agg[:], msgs[:, cs, 0:cfg.hid], cmat[:],
                        start=(cs == slots[0]), stop=False)
                # self-loop term: lhsT = stage tile (dinv*h), rhs = diag(dinv)
                dmat = c_p.tile([128, 128], bf16)
                nc.vector.tensor_scalar(
                    dmat[:], iota_ps[:],
                    partcol_sb[:, 0:1],
                    dinv_sb[:, p:p + 1],
                    mybir.AluOpType.is_equal,
                    mybir.AluOpType.mult)
                nc.tensor.matmul(
                    agg[:], stage[:, p, 0:cfg.hid], dmat[:],
                    start=False, stop=True)
                relu = relu_p.tile([cfg.hid, 128], bf16)
                nc.scalar.activation(
                    relu[:], agg[:], mybir.ActivationFunctionType.Relu,
                    bias=b1_sb[:])
                o2 = o2_ps.tile([cfg.out_dim, 128], f32)
                nc.tensor.matmul(o2[:], wfcT_sb[:], relu[:],
                                 start=True, stop=True)
                if i == 0:
                    ostage = ost_p.tile([cfg.out_dim, G * 128], f32)
                nc.scalar.activation(
                    ostage[:, i * 128:(i + 1) * 128], o2[:],
                    mybir.ActivationFunctionType.Identity, bias=bfc_sb[:])
            nc.sync.dma_start(
                out_d[:, g * G * 128:(g + 1) * G * 128], ostage[:])

    nc.compile()
    return nc


# ----------------------------------------------------------------------------
# Entry points.
# ----------------------------------------------------------------------------

_CACHE = {}


def _get_compiled(edges):
    import hashlib
    e = np.ascontiguousarray(np.asarray(edges, dtype=np.int64))
    key = (e.shape, hashlib.sha1(e.tobytes()).hexdigest())
    if key not in _CACHE:
        cfg, plan = _plan(e)
        nc = _build_module(cfg, plan)
        _CACHE[key] = (cfg, plan, nc)
    return _CACHE[key]


def _run(X, edges, W1, b1, Wfc, bfc, trace=False):
    from concourse.bass_utils import run_bass_kernel_spmd

    cfg, plan, nc = _get_compiled(edges)
    in_maps = _make_inputs(X, W1, b1, Wfc, bfc, cfg, plan)
    res = run_bass_kernel_spmd(
        nc, in_maps, core_ids=list(range(cfg.n_cores)), trace=trace)

    # Device output column (c, p, lane) -> node via stage_tile mapping.
    nt = cfg.nt
    full = np.concatenate([res.results[c]["out"] for c in range(cfg.n_cores)],
                          axis=1)                      # [40, n_slots]
    node_tile = plan["node_tile"]
    node_lane = plan["node_lane"]
    stage_tile = plan["stage_tile"]
    inv_stage = np.empty_like(stage_tile)
    for c in range(cfg.n_cores):
        inv_stage[c, stage_tile[c]] = np.arange(nt)
    n_core = node_tile // nt
    n_p = inv_stage[n_core, node_tile % nt]
    col = n_core * (nt * 128) + n_p * 128 + node_lane
    out = full[:, col].T.astype(np.float32)
    out = np.ascontiguousarray(out)
    return out, res


def kernel(X, edges, W1, b1, Wfc, bfc):
    out, _ = _run(np.asarray(X, dtype=np.float32), np.asarray(edges),
                  np.asarray(W1, dtype=np.float32),
                  np.asarray(b1, dtype=np.float32),
                  np.asarray(Wfc, dtype=np.float32),
                  np.asarray(bfc, dtype=np.float32))
    return out


# revision 21
# speedup vs baseline: 2.1904x; 1.2592x over previous
"""Trainium2 Bass kernel for a 2-layer GCN (GCNConv -> ReLU -> Linear).

Math (matching the PyG-style reference):
    deg  = in_degree(dst) + 1 (self loops), dinv = deg^-1/2
    h    = X @ W1                                  [N, 64]
    agg[d] = dinv[d] * sum_{e:(s->d)} dinv[s]*h[s] (+ self loop)   [N, 64]
    out  = relu(agg + b1) @ Wfc.T + bfc            [N, 40]

Distribution over 8 NeuronCores (graph/data parallel):
  - Nodes are re-labeled into 392 "tiles" of 128 slots (balanced so each
    (core, tile) has nearly equal lo/hi in-edge counts), 49 tiles per core.
    Each core computes h' = dinv*h for its 6272 slots (X @ W1 on the tensor
    engine) in two halves; each half is AllGather'd as soon as it is ready
    (two collectives, overlapped with compute / descriptor generation).
  - Each core aggregates the edges whose destination it owns: a bulk
    SWDGE dma_gather fetches h'[src] rows (256 B each) into SBUF in
    edge-major layout; destinations are scatter-added via one-hot matmuls
    (lhsT = gathered messages [128 edges, 64], rhs = C [128 edges, 128 dst]
    with C[j, d] = dinv_dst[j] * (dst_loc[j] == d)) accumulated in PSUM.
    C is built on the vector engine from a PSUM-resident iota (PSUM source
    keeps the op out of the 2-port DVE perf modes, which would otherwise
    lock GpSimd out of the shared SBUF port during SWDGE desc-gen).
  - Self loops never enter the gather stream: a per-tile diagonal matmul
    (lhsT = local h' tile, rhs = diag(dinv)) adds dinv^2*h directly.
  - relu(agg + b1) is fused into the PSUM->SBUF eviction on the scalar
    engine, the second layer is one small matmul per tile, and the bfc bias
    rides the final eviction.  The host un-permutes the [40, slots] outputs.

dma_gather indices are int16, so the table is built as TWO AllGather
outputs: half 0 = tiles 0..24 of every core (25600 rows), half 1 = tiles
25..48 (24576 rows); each tile's edges are packed into k0[t] half-0 chunks
followed by k1[t] half-1 chunks (pad lanes: idx=0, dst_loc=-1, dinv_dst=0).
The per-tile chunk-count profile (k0, k1) is shared by all cores (SPMD);
each core's tiles are assigned to profile slots by sorted edge counts.
"""

import numpy as np

# ----------------------------------------------------------------------------
# Problem configuration (hardcoded; kernel.py must be self-contained).
# ----------------------------------------------------------------------------
N_NODES = 50000
N_EDGES = 800000
IN_DIM = 512
HID = 64
OUT_DIM = 40
N_CORES = 8
TILES_PER_CORE = 49
GROUP = 7                    # tiles per gather/staging group
HALF_T = 32                  # tiles 0..HALF_T-1 are table half 0 (32*8*128 = 32768 rows = int16 limit)
SCALAR_CBUILD_MOD = 3        # every 3rd C-matrix build goes to ScalarE

GATHER_MAX_CHUNKS = 18
GATHER_SINGLE_PACKET = False


class Cfg:
    def __init__(self, n_nodes, in_dim, hid, out_dim, n_cores, k0, k1):
        self.n_nodes = n_nodes
        self.in_dim = in_dim
        self.hid = hid
        self.out_dim = out_dim
        self.n_cores = n_cores
        self.nt = TILES_PER_CORE
        self.group = GROUP
        self.n_groups = self.nt // self.group
        self.slots_per_core = self.nt * 128
        self.n_tiles = n_cores * self.nt
        self.n_slots = self.n_tiles * 128
        self.half0_rows = n_cores * HALF_T * 128       # 25600
        self.half1_rows = n_cores * (self.nt - HALF_T) * 128   # 24576
        self.k0 = k0                  # per-tile half-0 chunk counts [nt]
        self.k1 = k1                  # per-tile half-1 chunk counts [nt]
        self.n_chunks = int(np.sum(k0) + np.sum(k1))
        self.kd = in_dim // 128
        assert in_dim % 128 == 0
        assert self.half0_rows <= 32768 and self.half1_rows <= 32768


def _slot_of(tile, lane, n_cores):
    """Global table row of (global tile, lane) under the two-half layout."""
    core = tile // TILES_PER_CORE
    t = tile % TILES_PER_CORE
    half0 = n_cores * HALF_T * 128
    return np.where(
        t < HALF_T,
        core * (HALF_T * 128) + t * 128 + lane,
        half0 + core * ((TILES_PER_CORE - HALF_T) * 128) + (t - HALF_T) * 128
        + lane,
    )


# ----------------------------------------------------------------------------
# Host-side graph preprocessing (index/layout work only; all feature math
# runs on the device).
# ----------------------------------------------------------------------------

def _plan(edges):
    n_nodes, n_cores = N_NODES, N_CORES
    nt = TILES_PER_CORE
    n_tiles = n_cores * nt

    src = np.asarray(edges[0], dtype=np.int64)
    dst = np.asarray(edges[1], dtype=np.int64)
    deg = np.bincount(dst, minlength=n_nodes).astype(np.int64) + 1
    dinv = (1.0 / np.sqrt(deg.astype(np.float64))).astype(np.float32)

    # Snake binpack nodes (by degree, desc) into n_tiles bins of <=128 slots.
    order = np.argsort(-deg, kind="stable")
    rounds = np.arange(n_nodes) // n_tiles
    pos = np.arange(n_nodes) % n_tiles
    tile_of = np.where(rounds % 2 == 0, pos, n_tiles - 1 - pos)
    assert rounds.max() < 128, "more than 128 slots per tile"
    node_tile = np.empty(n_nodes, dtype=np.int64)
    node_lane = np.empty(n_nodes, dtype=np.int64)
    node_tile[order] = tile_of
    node_lane[order] = rounds

    half0 = n_cores * HALF_T * 128
    d_tile = node_tile[dst]
    d_lane = node_lane[dst]
    d_core = d_tile // nt
    d_tloc = d_tile % nt

    # The AllGather table stores each core's tiles in PROFILE-SLOT order
    # (stage column p holds local tile sort_t[c, p]), so an edge's class
    # (which table half its source lives in) depends on the source tile's
    # profile slot — which depends on per-class counts.  Two rounds: round 1
    # assigns profile slots using real-tile classes; round 2 recomputes
    # classes/counts under that fixed assignment.
    s_core = node_tile[src] // nt
    s_tloc = node_tile[src] % nt
    prof_slot = np.tile(np.arange(nt), (n_cores, 1))
    sort_t = np.tile(np.arange(nt), (n_cores, 1))
    for _round in range(2):
        s_p = prof_slot[s_core, s_tloc]
        cls = (s_p >= HALF_T).astype(np.int64)
        cnt = np.zeros((n_cores, nt, 2), dtype=np.int64)
        np.add.at(cnt, (d_core, d_tloc, cls), 1)
        key = cnt[:, :, 0] * 100000 + cnt[:, :, 1]
        sort_t = np.argsort(-key, axis=1, kind="stable")   # [cores, nt]
        prof_slot = np.empty((n_cores, nt), dtype=np.int64)
        for c in range(n_cores):
            prof_slot[c, sort_t[c]] = np.arange(nt)

    s_p = prof_slot[s_core, s_tloc]
    cls = (s_p >= HALF_T).astype(np.int64)
    cnt = np.zeros((n_cores, nt, 2), dtype=np.int64)
    np.add.at(cnt, (d_core, d_tloc, cls), 1)
    cnt_sorted = np.take_along_axis(cnt, sort_t[:, :, None], axis=1)
    k0 = np.max((cnt_sorted[:, :, 0] + 127) // 128, axis=0).astype(np.int64)
    k1 = np.max((cnt_sorted[:, :, 1] + 127) // 128, axis=0).astype(np.int64)
    k0 = np.maximum(k0, 1)
    k1 = np.maximum(k1, 1)
    # profile slots 0..HALF_T-1 must be the half-0 chunk-heavy ones?  No —
    # halves are POSITIONS: table half 0 = profile slots < HALF_T of every
    # core.  (sort order only balances counts.)

    cfg = Cfg(n_nodes, IN_DIM, HID, OUT_DIM, n_cores, k0, k1)

    # Source table row of each edge (profile-slot based).
    s_row = _slot_of(s_core * nt + s_p, node_lane[src], n_cores)

    slot_dinv = np.zeros(n_cores * nt * 128, dtype=np.float32)
    slot_dinv[_slot_of(node_tile, node_lane, n_cores)] = dinv

    # Chunk-slot numbering (shared across cores): profile slots are laid out
    # group-major; within a group: all half-0 chunks (tile-major), then all
    # half-1 chunks.
    #   base0[g] = start of group g's chunk range
    c0_in_g = np.zeros(nt, dtype=np.int64)   # chunk offset of tile within grp
    c1_in_g = np.zeros(nt, dtype=np.int64)
    g_nch = np.zeros(cfg.n_groups, dtype=np.int64)
    g_nch0 = np.zeros(cfg.n_groups, dtype=np.int64)
    g_base = np.zeros(cfg.n_groups, dtype=np.int64)
    acc = 0
    for g in range(cfg.n_groups):
        ts = np.arange(g * GROUP, (g + 1) * GROUP)
        g_base[g] = acc
        off = 0
        for t in ts:
            c0_in_g[t] = off
            off += k0[t]
        g_nch0[g] = off
        for t in ts:
            c1_in_g[t] = off
            off += k1[t]
        g_nch[g] = off
        acc += off
    n_chunks = acc
    assert n_chunks == cfg.n_chunks

    # Rank edges within (core, profile slot, class).
    p_slot = prof_slot[d_core, d_tloc]
    ekey = ((d_core * nt + p_slot) * 2 + cls)
    sort_idx = np.argsort(ekey, kind="stable")
    ekey_s = ekey[sort_idx]
    starts = np.searchsorted(ekey_s, np.arange(n_cores * nt * 2))
    rank = np.arange(len(ekey_s)) - starts[ekey_s]

    es_core = d_core[sort_idx]
    es_pslot = p_slot[sort_idx]
    es_cls = cls[sort_idx]
    es_g = es_pslot // GROUP
    j_chunk = rank >> 7
    lane = rank & 127
    fc = np.where(
        es_cls == 0,
        g_base[es_g] + c0_in_g[es_pslot] + j_chunk,
        g_base[es_g] + c1_in_g[es_pslot] + j_chunk,
    )
    assert (j_chunk < np.where(es_cls == 0, k0[es_pslot], k1[es_pslot])).all()

    idx16 = np.zeros((n_cores, n_chunks, 128), dtype=np.int16)
    dstloc = np.full((n_cores, n_chunks, 128), -1.0, dtype=np.float32)
    dinvdst = np.zeros((n_cores, n_chunks, 128), dtype=np.float32)

    s_rel = np.where(es_cls == 0, s_row[sort_idx],
                     s_row[sort_idx] - half0).astype(np.int16)
    idx16[es_core, fc, lane] = s_rel
    dstloc[es_core, fc, lane] = d_lane[sort_idx].astype(np.float32)
    dinvdst[es_core, fc, lane] = dinv[dst[sort_idx]]

    # Wrap gather indices: per (group, class) region, list position s ->
    # partition s%16, column s//16; replicated across the 8 q7 cores.
    n_idx_cols = n_chunks * 8
    idx_wrapped = np.zeros((n_cores, 128, n_idx_cols), dtype=np.int16)
    for g in range(cfg.n_groups):
        for lo, hi in ((0, g_nch0[g]), (g_nch0[g], g_nch[g])):
            nch = hi - lo
            if nch == 0:
                continue
            fc0 = g_base[g] + lo
            flat = idx16[:, fc0:fc0 + nch, :].reshape(n_cores, nch * 128)
            wrapped = flat.reshape(n_cores, nch * 8, 16).transpose(0, 2, 1)
            c0 = fc0 * 8
            idx_wrapped[:, :16, c0:c0 + nch * 8] = wrapped
    idx_wrapped[:, 16:, :] = np.tile(idx_wrapped[:, :16, :], (1, 7, 1))

    # Per-profile-slot dinv columns (for h' scaling and diag matmuls): the
    # lanes of (core, profile slot p) are those of its assigned local tile.
    # Phase-1 stage is laid out in PROFILE-SLOT order so that gather chunk /
    # matmul structure is SPMD-uniform; the AllGather table rows follow the
    # same order, and node_to_slot already accounts for it via _slot_of on
    # REAL tiles... stage column p holds local tile sort_t[c, p].
    stage_tile = sort_t                                   # [cores, nt]
    # dinv per (core, profile slot, lane):
    dinv_ps = np.zeros((n_cores, 128, nt), dtype=np.float32)
    for c in range(n_cores):
        for p in range(nt):
            t = stage_tile[c, p]
            gt = c * nt + t
            sl = _slot_of(np.full(128, gt), np.arange(128), n_cores)
            dinv_ps[c, :, p] = slot_dinv[sl]

    plan = dict(
        node_tile=node_tile, node_lane=node_lane,
        slot_dinv=slot_dinv, stage_tile=stage_tile,
        idx_wrapped=idx_wrapped,
        dstloc=dstloc.transpose(0, 2, 1).copy(),
        dinvdst=dinvdst.transpose(0, 2, 1).copy(),
        dinv_ps=dinv_ps,
        g_nch=g_nch, g_nch0=g_nch0, g_base=g_base,
        c0_in_g=c0_in_g, c1_in_g=c1_in_g,
    )
    return cfg, plan


def _make_inputs(X, W1, b1, Wfc, bfc, cfg, plan):
    """Build the 8 per-core input dicts for run_bass_kernel_spmd."""
    import ml_dtypes
    bf16 = ml_dtypes.bfloat16
    nt = cfg.nt

    # X rows arranged per (core, PROFILE slot, lane): stage column p of core
    # c holds local tile stage_tile[c, p].
    Xp = np.zeros((cfg.n_slots, cfg.in_dim), dtype=np.float32)
    # destination row for node n: core*nt*128 + prof_slot... we need X in the
    # ORDER phase 1 consumes it: xt[:, k, p*128+lane] = X[node at (c,p,lane)]
    node_tile = plan["node_tile"]
    node_lane = plan["node_lane"]
    stage_tile = plan["stage_tile"]
    inv_stage = np.empty_like(stage_tile)
    for c in range(cfg.n_cores):
        inv_stage[c, stage_tile[c]] = np.arange(nt)
    n_core = node_tile // nt
    n_tloc = node_tile % nt
    n_p = inv_stage[n_core, n_tloc]
    xrow = n_core * (nt * 128) + n_p * 128 + node_lane
    Xp[xrow] = np.asarray(X, dtype=np.float32)

    W1r = (np.asarray(W1, dtype=np.float32)
           .reshape(cfg.kd, 128, cfg.hid).transpose(1, 0, 2)
           .reshape(128, cfg.kd * cfg.hid).astype(bf16))
    wfcT = np.ascontiguousarray(np.asarray(Wfc, dtype=np.float32).T).astype(bf16)
    b1c = np.asarray(b1, dtype=np.float32).reshape(cfg.hid, 1)
    bfcc = np.asarray(bfc, dtype=np.float32).reshape(cfg.out_dim, 1)
    iota = np.ascontiguousarray(
        np.tile(np.arange(128, dtype=np.float32), (128, 1)))
    partcol = np.arange(128, dtype=np.float32).reshape(128, 1)

    s = cfg.slots_per_core
    in_maps = []
    for c in range(cfg.n_cores):
        xt = np.ascontiguousarray(Xp[c * s:(c + 1) * s].T).astype(bf16)
        in_maps.append({
            "xt": xt,
            "w1": W1r,
            "wfcT": wfcT,
            "b1": b1c,
            "bfc": bfcc,
            "iota": iota,
            "partcol": partcol,
            "dinv_sb": np.ascontiguousarray(plan["dinv_ps"][c]),
            "idx": plan["idx_wrapped"][c],
            "dstloc": plan["dstloc"][c],
            "dinvdst": plan["dinvdst"][c],
            "negdinvdst": np.ascontiguousarray(-plan["dinvdst"][c]),
        })
    return in_maps


# ----------------------------------------------------------------------------
# Device kernel.
# ----------------------------------------------------------------------------

def _build_module(cfg, plan):
    import concourse.bass as bass
    import concourse.bacc as bacc
    import concourse.mybir as mybir
    import concourse.tile as tile
    from contextlib import ExitStack

    f32 = mybir.dt.float32
    bf16 = mybir.dt.bfloat16
    i16 = mybir.dt.int16
    S = cfg.slots_per_core
    G = cfg.group
    nt = cfg.nt
    n_chunks = cfg.n_chunks
    n_idx_cols = n_chunks * 8
    g_nch = plan["g_nch"]
    g_nch0 = plan["g_nch0"]
    g_base = plan["g_base"]
    c0_in_g = plan["c0_in_g"]
    c1_in_g = plan["c1_in_g"]
    k0, k1 = cfg.k0, cfg.k1
    H0T = HALF_T                 # tiles in half 0
    H1T = nt - HALF_T
    NCHG_MAX = int(np.max(g_nch))

    nc = bacc.Bacc("TRN2", target_bir_lowering=False, debug=False,
                   num_devices=cfg.n_cores, num_swdge_queues=4)

    xt_d = nc.dram_tensor("xt", [cfg.in_dim, S], bf16, kind="ExternalInput")
    w1_d = nc.dram_tensor("w1", [128, cfg.kd * cfg.hid], bf16,
                          kind="ExternalInput")
    wfcT_d = nc.dram_tensor("wfcT", [cfg.hid, cfg.out_dim], bf16,
                            kind="ExternalInput")
    b1_d = nc.dram_tensor("b1", [cfg.hid, 1], f32, kind="ExternalInput")
    bfc_d = nc.dram_tensor("bfc", [cfg.out_dim, 1], f32, kind="ExternalInput")
    iota_d = nc.dram_tensor("iota", [128, 128], f32, kind="ExternalInput")
    partcol_d = nc.dram_tensor("partcol", [128, 1], f32, kind="ExternalInput")
    dinv_d = nc.dram_tensor("dinv_sb", [128, nt], f32, kind="ExternalInput")
    idx_d = nc.dram_tensor("idx", [128, n_idx_cols], i16, kind="ExternalInput")
    dstloc_d = nc.dram_tensor("dstloc", [128, n_chunks], f32,
                              kind="ExternalInput")
    dinvdst_d = nc.dram_tensor("dinvdst", [128, n_chunks], f32,
                               kind="ExternalInput")
    negdinvdst_d = nc.dram_tensor("negdinvdst", [128, n_chunks], f32,
                                  kind="ExternalInput")
    out_d = nc.dram_tensor("out", [cfg.out_dim, S], f32, kind="ExternalOutput")

    with tile.TileContext(nc) as tc, ExitStack() as ctx:
        dram = ctx.enter_context(tc.tile_pool(name="dram", bufs=1,
                                              space="DRAM"))
        consts = ctx.enter_context(tc.tile_pool(name="consts", bufs=1))
        persist = ctx.enter_context(tc.tile_pool(name="persist", bufs=1))
        ag0_in = dram.tile([H0T * 128, 128], bf16)
        ag1_in = dram.tile([H1T * 128, 128], bf16)
        ag0_out = dram.tile([cfg.half0_rows, 128], bf16, addr_space="Shared")
        ag1_out = dram.tile([cfg.half1_rows, 128], bf16, addr_space="Shared")

        # iota lives in PSUM: a PSUM-source tensor_scalar cannot enter the
        # 2-port DVE perf modes, so the per-chunk C-matrix builds never grab
        # the shared SBUF port pair that SWDGE descriptor generation (the
        # dma_gather Q7 loop) needs — they'd otherwise serialize.
        iota_pp = ctx.enter_context(
            tc.tile_pool(name="iotapp", bufs=1, space="PSUM"))
        iota_ps = iota_pp.tile([128, 128], f32)

        iota_sb = consts.tile([128, 128], f32)
        partcol_sb = consts.tile([128, 1], f32)
        w1_sb = consts.tile([128, cfg.kd * cfg.hid], bf16)
        wfcT_sb = consts.tile([cfg.hid, cfg.out_dim], bf16)
        b1_sb = consts.tile([cfg.hid, 1], f32)
        bfc_sb = consts.tile([cfg.out_dim, 1], f32)
        dinv_sb = consts.tile([128, nt], f32)
        idx_sb = consts.tile([128, n_idx_cols], i16)
        dstloc_sb = consts.tile([128, n_chunks], f32)
        dinvdst_sb = consts.tile([128, n_chunks], f32)
        negdinvdst_sb = consts.tile([128, n_chunks], f32)
        stage = persist.tile([128, nt, 128], bf16)
        xt_sb = persist.tile([128, cfg.kd, S], bf16)

        # xt first (phase 1's critical input), then the rest.
        nc.sync.dma_start(
            xt_sb[:, :, 0:H0T * 128],
            xt_d[:, 0:H0T * 128].rearrange("(k p) s -> p k s", p=128))
        nc.sync.dma_start(
            xt_sb[:, :, H0T * 128:S],
            xt_d[:, H0T * 128:S].rearrange("(k p) s -> p k s", p=128))
        nc.sync.dma_start(iota_sb[:], iota_d[:])
        nc.sync.dma_start(partcol_sb[:], partcol_d[:])
        nc.sync.dma_start(w1_sb[:], w1_d[:])
        nc.sync.dma_start(wfcT_sb[:], wfcT_d[:])
        nc.sync.dma_start(b1_sb[:], b1_d[:])
        nc.sync.dma_start(bfc_sb[:], bfc_d[:])
        nc.sync.dma_start(dinv_sb[:], dinv_d[:])
        nc.sync.dma_start(idx_sb[:], idx_d[:])
        nc.sync.dma_start(dstloc_sb[:], dstloc_d[:])
        nc.sync.dma_start(dinvdst_sb[:], dinvdst_d[:])
        nc.sync.dma_start(negdinvdst_sb[:], negdinvdst_d[:])
        nc.vector.tensor_scalar_mul(iota_ps[:], iota_sb[:], 1.0)

        # ---- Phase 1: h' = dinv * (X @ W1), bf16 rows padded to 256 B ----
        # Computed in two halves; each half is AllGather'd as soon as ready.
        with tc.tile_pool(name="p1ps", bufs=2, space="PSUM") as p1ps:
            for half, (t0, t1, ag_in, ag_out, rows) in enumerate((
                    (0, H0T, ag0_in, ag0_out, cfg.half0_rows),
                    (H0T, nt, ag1_in, ag1_out, cfg.half1_rows))):
                for t in range(t0, t1):
                    ph = p1ps.tile([128, cfg.hid], f32)
                    for k in range(cfg.kd):
                        nc.tensor.matmul(
                            ph[:],
                            xt_sb[:, k, t * 128:(t + 1) * 128],
                            w1_sb[:, k * cfg.hid:(k + 1) * cfg.hid],
                            start=(k == 0), stop=(k == cfg.kd - 1))
                    nc.vector.tensor_scalar_mul(
                        stage[:, t, 0:cfg.hid], ph[:],
                        dinv_sb[:, t:t + 1])
                nc.sync.dma_start(
                    ag_in[:].rearrange("(t p) e -> p t e", p=128),
                    stage[:, t0:t1, :])
                nc.gpsimd.collective_compute(
                    "AllGather",
                    mybir.AluOpType.bypass,
                    ins=[ag_in.opt()],
                    outs=[ag_out.opt()],
                    replica_groups=[list(range(cfg.n_cores))],
                )

        # ---- Phase 2: gather + one-hot scatter matmuls + layer 2 ----
        msgs_p = ctx.enter_context(tc.tile_pool(name="msgs", bufs=2))
        c_p = ctx.enter_context(tc.tile_pool(name="cmat", bufs=12))
        relu_p = ctx.enter_context(tc.tile_pool(name="relu", bufs=3))
        ost_p = ctx.enter_context(tc.tile_pool(name="ost", bufs=2))
        agg_ps = ctx.enter_context(
            tc.tile_pool(name="aggps", bufs=4, space="PSUM"))
        o2_ps = ctx.enter_context(
            tc.tile_pool(name="o2ps", bufs=2, space="PSUM"))

        GMAX = GATHER_MAX_CHUNKS
        gather_call = 0

        for g in range(cfg.n_groups):
            nchg = int(g_nch[g])
            nch0 = int(g_nch0[g])
            base = int(g_base[g])
            msgs = msgs_p.tile([128, NCHG_MAX, 128], bf16)
            col0 = base * 8
            for r0, r1, tbl in ((0, nch0, ag0_out[:, :]),
                                (nch0, nchg, ag1_out[:, :])):
                cs0 = r0
                while cs0 < r1:
                    nch = min(GMAX, r1 - cs0)
                    nc.gpsimd.dma_gather(
                        msgs[:, cs0:cs0 + nch, :], tbl,
                        idx_sb[:, col0 + cs0 * 8: col0 + (cs0 + nch) * 8],
                        nch * 128, nch * 128, 128,
                        single_packet=GATHER_SINGLE_PACKET,
                        queue_num=gather_call % 4)
                    gather_call += 1
                    cs0 += nch

            for i in range(G):
                p = g * G + i
                agg = agg_ps.tile([cfg.hid, 128], f32)
                # self-loop term first (no gather dependency): lhsT = stage
                # tile (dinv*h), rhs = diag(dinv).
                dmat = c_p.tile([128, 128], bf16)
                nc.vector.tensor_scalar(
                    dmat[:], iota_ps[:],
                    partcol_sb[:, 0:1],
                    dinv_sb[:, p:p + 1],
                    mybir.AluOpType.is_equal,
                    mybir.AluOpType.mult)
                nc.tensor.matmul(
                    agg[:], stage[:, p, 0:cfg.hid], dmat[:],
                    start=True, stop=False)
                slots = ([int(c0_in_g[p]) + j for j in range(int(k0[p]))]
                         + [int(c1_in_g[p]) + j for j in range(int(k1[p]))])
                for jj, cs in enumerate(slots):
                    gc = base + cs
                    cmat = c_p.tile([128, 128], bf16)
                    if gc % SCALAR_CBUILD_MOD == 0:
                        # ScalarE build: |dstloc - iota| -> relu(dinv*(1-t))
                        tabs = c_p.tile([128, 128], bf16)
                        nc.scalar.activation(
                            tabs[:], iota_sb[:],
                            mybir.ActivationFunctionType.Abs,
                            bias=dstloc_sb[:, gc:gc + 1], scale=-1.0)
                        nc.scalar.activation(
                            cmat[:], tabs[:],
                            mybir.ActivationFunctionType.Relu,
                            bias=dinvdst_sb[:, gc:gc + 1],
                            scale=negdinvdst_sb[:, gc:gc + 1])
                    else:
                        nc.vector.tensor_scalar(
                            cmat[:], iota_ps[:],
                            dstloc_sb[:, gc:gc + 1],
                            dinvdst_sb[:, gc:gc + 1],
                            mybir.AluOpType.is_equal,
                            mybir.AluOpType.mult)
                    nc.tensor.matmul(
                        agg[:], msgs[:, cs, 0:cfg.hid], cmat[:],
                        start=False, stop=(jj == len(slots) - 1))
                relu = relu_p.tile([cfg.hid, 128], bf16)
                nc.scalar.activation(
                    relu[:], agg[:], mybir.ActivationFunctionType.Relu,
                    bias=b1_sb[:])
                o2 = o2_ps.tile([cfg.out_dim, 128], f32)
                nc.tensor.matmul(o2[:], wfcT_sb[:], relu[:],
                                 start=True, stop=True)
                if i == 0:
                    ostage = ost_p.tile([cfg.out_dim, G * 128], f32)
                nc.scalar.activation(
                    ostage[:, i * 128:(i + 1) * 128], o2[:],
                    mybir.ActivationFunctionType.Identity, bias=bfc_sb[:])
            nc.sync.dma_start(
                out_d[:, g * G * 128:(g + 1) * G * 128], ostage[:])

    nc.compile()
    return nc


# ----------------------------------------------------------------------------
# Entry points.
# ----------------------------------------------------------------------------

_CACHE = {}


def _get_compiled(edges):
    import hashlib
    e = np.ascontiguousarray(np.asarray(edges, dtype=np.int64))
    key = (e.shape, hashlib.sha1(e.tobytes()).hexdigest())
    if key not in _CACHE:
        cfg, plan = _plan(e)
        nc = _build_module(cfg, plan)
        _CACHE[key] = (cfg, plan, nc)
    return _CACHE[key]


def _run(X, edges, W1, b1, Wfc, bfc, trace=False):
    from concourse.bass_utils import run_bass_kernel_spmd

    cfg, plan, nc = _get_compiled(edges)
    in_maps = _make_inputs(X, W1, b1, Wfc, bfc, cfg, plan)
    res = run_bass_kernel_spmd(
        nc, in_maps, core_ids=list(range(cfg.n_cores)), trace=trace)

    # Device output column (c, p, lane) -> node via stage_tile mapping.
    nt = cfg.nt
    full = np.concatenate([res.results[c]["out"] for c in range(cfg.n_cores)],
                          axis=1)                      # [40, n_slots]
    node_tile = plan["node_tile"]
    node_lane = plan["node_lane"]
    stage_tile = plan["stage_tile"]
    inv_stage = np.empty_like(stage_tile)
    for c in range(cfg.n_cores):
        inv_stage[c, stage_tile[c]] = np.arange(nt)
    n_core = node_tile // nt
    n_p = inv_stage[n_core, node_tile % nt]
    col = n_core * (nt * 128) + n_p * 128 + node_lane
    out = full[:, col].T.astype(np.float32)
    out = np.ascontiguousarray(out)
    return out, res


def kernel(X, edges, W1, b1, Wfc, bfc):
    out, _ = _run(np.asarray(X, dtype=np.float32), np.asarray(edges),
                  np.asarray(W1, dtype=np.float32),
                  np.asarray(b1, dtype=np.float32),
                  np.asarray(Wfc, dtype=np.float32),
                  np.asarray(bfc, dtype=np.float32))
    return out


# revision 22
# speedup vs baseline: 2.3670x; 1.0807x over previous
"""Trainium2 Bass kernel for a 2-layer GCN (GCNConv -> ReLU -> Linear).

Math (matching the PyG-style reference):
    deg  = in_degree(dst) + 1 (self loops), dinv = deg^-1/2
    h    = X @ W1                                  [N, 64]
    agg[d] = dinv[d] * sum_{e:(s->d)} dinv[s]*h[s] (+ self loop)   [N, 64]
    out  = relu(agg + b1) @ Wfc.T + bfc            [N, 40]

Distribution over 8 NeuronCores (graph/data parallel):
  - Nodes are re-labeled into 392 "tiles" of 128 slots (balanced so each
    (core, tile) has nearly equal lo/hi in-edge counts), 49 tiles per core.
    Each core computes h' = dinv*h for its 6272 slots (X @ W1 on the tensor
    engine) in two halves; each half is AllGather'd as soon as it is ready
    (two collectives, overlapped with compute / descriptor generation).
  - Each core aggregates the edges whose destination it owns: a bulk
    SWDGE dma_gather fetches h'[src] rows (256 B each) into SBUF in
    edge-major layout; destinations are scatter-added via one-hot matmuls
    (lhsT = gathered messages [128 edges, 64], rhs = C [128 edges, 128 dst]
    with C[j, d] = dinv_dst[j] * (dst_loc[j] == d)) accumulated in PSUM.
    C is built on the vector engine from a PSUM-resident iota (PSUM source
    keeps the op out of the 2-port DVE perf modes, which would otherwise
    lock GpSimd out of the shared SBUF port during SWDGE desc-gen).
  - Self loops never enter the gather stream: a per-tile diagonal matmul
    (lhsT = local h' tile, rhs = diag(dinv)) adds dinv^2*h directly.
  - relu(agg + b1) is fused into the PSUM->SBUF eviction on the scalar
    engine, the second layer is one small matmul per tile, and the bfc bias
    rides the final eviction.  The host un-permutes the [40, slots] outputs.

dma_gather indices are int16, so the table is built as TWO AllGather
outputs: half 0 = tiles 0..24 of every core (25600 rows), half 1 = tiles
25..48 (24576 rows); each tile's edges are packed into k0[t] half-0 chunks
followed by k1[t] half-1 chunks (pad lanes: idx=0, dst_loc=-1, dinv_dst=0).
The per-tile chunk-count profile (k0, k1) is shared by all cores (SPMD);
each core's tiles are assigned to profile slots by sorted edge counts.
"""

import numpy as np

# ----------------------------------------------------------------------------
# Problem configuration (hardcoded; kernel.py must be self-contained).
# ----------------------------------------------------------------------------
N_NODES = 50000
N_EDGES = 800000
IN_DIM = 512
HID = 64
OUT_DIM = 40
N_CORES = 8
TILES_PER_CORE = 49
GROUP = 7                    # tiles per gather/staging group
HALF_T = 17                  # tiles 0..HALF_T-1 are table half 0 (small half
                             # first: its AllGather completes early so class-0
                             # gathers start while the big half is still in
                             # flight; 32*8*128 = 32768 = int16 limit binds
                             # the big half)
SCALAR_CBUILD_MOD = 3        # every 3rd C-matrix build goes to ScalarE

GATHER_MAX_CHUNKS = 18
GATHER_SINGLE_PACKET = False


class Cfg:
    def __init__(self, n_nodes, in_dim, hid, out_dim, n_cores, k0, k1):
        self.n_nodes = n_nodes
        self.in_dim = in_dim
        self.hid = hid
        self.out_dim = out_dim
        self.n_cores = n_cores
        self.nt = TILES_PER_CORE
        self.group = GROUP
        self.n_groups = self.nt // self.group
        self.slots_per_core = self.nt * 128
        self.n_tiles = n_cores * self.nt
        self.n_slots = self.n_tiles * 128
        self.half0_rows = n_cores * HALF_T * 128       # 25600
        self.half1_rows = n_cores * (self.nt - HALF_T) * 128   # 24576
        self.k0 = k0                  # per-tile half-0 chunk counts [nt]
        self.k1 = k1                  # per-tile half-1 chunk counts [nt]
        self.n_chunks = int(np.sum(k0) + np.sum(k1))
        self.kd = in_dim // 128
        assert in_dim % 128 == 0
        assert self.half0_rows <= 32768 and self.half1_rows <= 32768


def _slot_of(tile, lane, n_cores):
    """Global table row of (global tile, lane) under the two-half layout."""
    core = tile // TILES_PER_CORE
    t = tile % TILES_PER_CORE
    half0 = n_cores * HALF_T * 128
    return np.where(
        t < HALF_T,
        core * (HALF_T * 128) + t * 128 + lane,
        half0 + core * ((TILES_PER_CORE - HALF_T) * 128) + (t - HALF_T) * 128
        + lane,
    )


# ----------------------------------------------------------------------------
# Host-side graph preprocessing (index/layout work only; all feature math
# runs on the device).
# ----------------------------------------------------------------------------

def _plan(edges):
    n_nodes, n_cores = N_NODES, N_CORES
    nt = TILES_PER_CORE
    n_tiles = n_cores * nt

    src = np.asarray(edges[0], dtype=np.int64)
    dst = np.asarray(edges[1], dtype=np.int64)
    deg = np.bincount(dst, minlength=n_nodes).astype(np.int64) + 1
    dinv = (1.0 / np.sqrt(deg.astype(np.float64))).astype(np.float32)

    # Snake binpack nodes (by degree, desc) into n_tiles bins of <=128 slots.
    order = np.argsort(-deg, kind="stable")
    rounds = np.arange(n_nodes) // n_tiles
    pos = np.arange(n_nodes) % n_tiles
    tile_of = np.where(rounds % 2 == 0, pos, n_tiles - 1 - pos)
    assert rounds.max() < 128, "more than 128 slots per tile"
    node_tile = np.empty(n_nodes, dtype=np.int64)
    node_lane = np.empty(n_nodes, dtype=np.int64)
    node_tile[order] = tile_of
    node_lane[order] = rounds

    half0 = n_cores * HALF_T * 128
    d_tile = node_tile[dst]
    d_lane = node_lane[dst]
    d_core = d_tile // nt
    d_tloc = d_tile % nt

    # The AllGather table stores each core's tiles in PROFILE-SLOT order
    # (stage column p holds local tile sort_t[c, p]), so an edge's class
    # (which table half its source lives in) depends on the source tile's
    # profile slot — which depends on per-class counts.  Two rounds: round 1
    # assigns profile slots using real-tile classes; round 2 recomputes
    # classes/counts under that fixed assignment.
    s_core = node_tile[src] // nt
    s_tloc = node_tile[src] % nt
    prof_slot = np.tile(np.arange(nt), (n_cores, 1))
    sort_t = np.tile(np.arange(nt), (n_cores, 1))
    for _round in range(2):
        s_p = prof_slot[s_core, s_tloc]
        cls = (s_p >= HALF_T).astype(np.int64)
        cnt = np.zeros((n_cores, nt, 2), dtype=np.int64)
        np.add.at(cnt, (d_core, d_tloc, cls), 1)
        key = cnt[:, :, 0] * 100000 + cnt[:, :, 1]
        sort_t = np.argsort(-key, axis=1, kind="stable")   # [cores, nt]
        prof_slot = np.empty((n_cores, nt), dtype=np.int64)
        for c in range(n_cores):
            prof_slot[c, sort_t[c]] = np.arange(nt)

    s_p = prof_slot[s_core, s_tloc]
    cls = (s_p >= HALF_T).astype(np.int64)
    cnt = np.zeros((n_cores, nt, 2), dtype=np.int64)
    np.add.at(cnt, (d_core, d_tloc, cls), 1)
    cnt_sorted = np.take_along_axis(cnt, sort_t[:, :, None], axis=1)
    k0 = np.max((cnt_sorted[:, :, 0] + 127) // 128, axis=0).astype(np.int64)
    k1 = np.max((cnt_sorted[:, :, 1] + 127) // 128, axis=0).astype(np.int64)
    k0 = np.maximum(k0, 1)
    k1 = np.maximum(k1, 1)
    # profile slots 0..HALF_T-1 must be the half-0 chunk-heavy ones?  No —
    # halves are POSITIONS: table half 0 = profile slots < HALF_T of every
    # core.  (sort order only balances counts.)

    cfg = Cfg(n_nodes, IN_DIM, HID, OUT_DIM, n_cores, k0, k1)

    # Source table row of each edge (profile-slot based).
    s_row = _slot_of(s_core * nt + s_p, node_lane[src], n_cores)

    slot_dinv = np.zeros(n_cores * nt * 128, dtype=np.float32)
    slot_dinv[_slot_of(node_tile, node_lane, n_cores)] = dinv

    # Chunk-slot numbering (shared across cores): profile slots are laid out
    # group-major; within a group: all half-0 chunks (tile-major), then all
    # half-1 chunks.
    #   base0[g] = start of group g's chunk range
    c0_in_g = np.zeros(nt, dtype=np.int64)   # chunk offset of tile within grp
    c1_in_g = np.zeros(nt, dtype=np.int64)
    g_nch = np.zeros(cfg.n_groups, dtype=np.int64)
    g_nch0 = np.zeros(cfg.n_groups, dtype=np.int64)
    g_base = np.zeros(cfg.n_groups, dtype=np.int64)
    acc = 0
    for g in range(cfg.n_groups):
        ts = np.arange(g * GROUP, (g + 1) * GROUP)
        g_base[g] = acc
        off = 0
        for t in ts:
            c0_in_g[t] = off
            off += k0[t]
        g_nch0[g] = off
        for t in ts:
            c1_in_g[t] = off
            off += k1[t]
        g_nch[g] = off
        acc += off
    n_chunks = acc
    assert n_chunks == cfg.n_chunks

    # Rank edges within (core, profile slot, class).
    p_slot = prof_slot[d_core, d_tloc]
    ekey = ((d_core * nt + p_slot) * 2 + cls)
    sort_idx = np.argsort(ekey, kind="stable")
    ekey_s = ekey[sort_idx]
    starts = np.searchsorted(ekey_s, np.arange(n_cores * nt * 2))
    rank = np.arange(len(ekey_s)) - starts[ekey_s]

    es_core = d_core[sort_idx]
    es_pslot = p_slot[sort_idx]
    es_cls = cls[sort_idx]
    es_g = es_pslot // GROUP
    j_chunk = rank >> 7
    lane = rank & 127
    fc = np.where(
        es_cls == 0,
        g_base[es_g] + c0_in_g[es_pslot] + j_chunk,
        g_base[es_g] + c1_in_g[es_pslot] + j_chunk,
    )
    assert (j_chunk < np.where(es_cls == 0, k0[es_pslot], k1[es_pslot])).all()

    idx16 = np.zeros((n_cores, n_chunks, 128), dtype=np.int16)
    dstloc = np.full((n_cores, n_chunks, 128), -1.0, dtype=np.float32)
    dinvdst = np.zeros((n_cores, n_chunks, 128), dtype=np.float32)

    s_rel = np.where(es_cls == 0, s_row[sort_idx],
                     s_row[sort_idx] - half0).astype(np.int16)
    idx16[es_core, fc, lane] = s_rel
    dstloc[es_core, fc, lane] = d_lane[sort_idx].astype(np.float32)
    dinvdst[es_core, fc, lane] = dinv[dst[sort_idx]]

    # Wrap gather indices: per (group, class) region, list position s ->
    # partition s%16, column s//16; replicated across the 8 q7 cores.
    n_idx_cols = n_chunks * 8
    idx_wrapped = np.zeros((n_cores, 128, n_idx_cols), dtype=np.int16)
    for g in range(cfg.n_groups):
        for lo, hi in ((0, g_nch0[g]), (g_nch0[g], g_nch[g])):
            nch = hi - lo
            if nch == 0:
                continue
            fc0 = g_base[g] + lo
            flat = idx16[:, fc0:fc0 + nch, :].reshape(n_cores, nch * 128)
            wrapped = flat.reshape(n_cores, nch * 8, 16).transpose(0, 2, 1)
            c0 = fc0 * 8
            idx_wrapped[:, :16, c0:c0 + nch * 8] = wrapped
    idx_wrapped[:, 16:, :] = np.tile(idx_wrapped[:, :16, :], (1, 7, 1))

    # Per-profile-slot dinv columns (for h' scaling and diag matmuls): the
    # lanes of (core, profile slot p) are those of its assigned local tile.
    # Phase-1 stage is laid out in PROFILE-SLOT order so that gather chunk /
    # matmul structure is SPMD-uniform; the AllGather table rows follow the
    # same order, and node_to_slot already accounts for it via _slot_of on
    # REAL tiles... stage column p holds local tile sort_t[c, p].
    stage_tile = sort_t                                   # [cores, nt]
    # dinv per (core, profile slot, lane):
    dinv_ps = np.zeros((n_cores, 128, nt), dtype=np.float32)
    for c in range(n_cores):
        for p in range(nt):
            t = stage_tile[c, p]
            gt = c * nt + t
            sl = _slot_of(np.full(128, gt), np.arange(128), n_cores)
            dinv_ps[c, :, p] = slot_dinv[sl]

    plan = dict(
        node_tile=node_tile, node_lane=node_lane,
        slot_dinv=slot_dinv, stage_tile=stage_tile,
        idx_wrapped=idx_wrapped,
        dstloc=dstloc.transpose(0, 2, 1).copy(),
        dinvdst=dinvdst.transpose(0, 2, 1).copy(),
        dinv_ps=dinv_ps,
        g_nch=g_nch, g_nch0=g_nch0, g_base=g_base,
        c0_in_g=c0_in_g, c1_in_g=c1_in_g,
    )
    return cfg, plan


def _make_inputs(X, W1, b1, Wfc, bfc, cfg, plan):
    """Build the 8 per-core input dicts for run_bass_kernel_spmd."""
    import ml_dtypes
    bf16 = ml_dtypes.bfloat16
    nt = cfg.nt

    # X rows arranged per (core, PROFILE slot, lane): stage column p of core
    # c holds local tile stage_tile[c, p].
    Xp = np.zeros((cfg.n_slots, cfg.in_dim), dtype=np.float32)
    # destination row for node n: core*nt*128 + prof_slot... we need X in the
    # ORDER phase 1 consumes it: xt[:, k, p*128+lane] = X[node at (c,p,lane)]
    node_tile = plan["node_tile"]
    node_lane = plan["node_lane"]
    stage_tile = plan["stage_tile"]
    inv_stage = np.empty_like(stage_tile)
    for c in range(cfg.n_cores):
        inv_stage[c, stage_tile[c]] = np.arange(nt)
    n_core = node_tile // nt
    n_tloc = node_tile % nt
    n_p = inv_stage[n_core, n_tloc]
    xrow = n_core * (nt * 128) + n_p * 128 + node_lane
    Xp[xrow] = np.asarray(X, dtype=np.float32)

    W1r = (np.asarray(W1, dtype=np.float32)
           .reshape(cfg.kd, 128, cfg.hid).transpose(1, 0, 2)
           .reshape(128, cfg.kd * cfg.hid).astype(bf16))
    wfcT = np.ascontiguousarray(np.asarray(Wfc, dtype=np.float32).T).astype(bf16)
    b1c = np.asarray(b1, dtype=np.float32).reshape(cfg.hid, 1)
    bfcc = np.asarray(bfc, dtype=np.float32).reshape(cfg.out_dim, 1)
    iota = np.ascontiguousarray(
        np.tile(np.arange(128, dtype=np.float32), (128, 1)))
    partcol = np.arange(128, dtype=np.float32).reshape(128, 1)

    s = cfg.slots_per_core
    in_maps = []
    for c in range(cfg.n_cores):
        xt = np.ascontiguousarray(Xp[c * s:(c + 1) * s].T).astype(bf16)
        in_maps.append({
            "xt": xt,
            "w1": W1r,
            "wfcT": wfcT,
            "b1": b1c,
            "bfc": bfcc,
            "iota": iota,
            "partcol": partcol,
            "dinv_sb": np.ascontiguousarray(plan["dinv_ps"][c]),
            "idx": plan["idx_wrapped"][c],
            "dstloc": plan["dstloc"][c],
            "dinvdst": plan["dinvdst"][c],
            "negdinvdst": np.ascontiguousarray(-plan["dinvdst"][c]),
        })
    return in_maps


# ----------------------------------------------------------------------------
# Device kernel.
# ----------------------------------------------------------------------------

def _build_module(cfg, plan):
    import concourse.bass as bass
    import concourse.bacc as bacc
    import concourse.mybir as mybir
    import concourse.tile as tile
    from contextlib import ExitStack

    f32 = mybir.dt.float32
    bf16 = mybir.dt.bfloat16
    i16 = mybir.dt.int16
    S = cfg.slots_per_core
    G = cfg.group
    nt = cfg.nt
    n_chunks = cfg.n_chunks
    n_idx_cols = n_chunks * 8
    g_nch = plan["g_nch"]
    g_nch0 = plan["g_nch0"]
    g_base = plan["g_base"]
    c0_in_g = plan["c0_in_g"]
    c1_in_g = plan["c1_in_g"]
    k0, k1 = cfg.k0, cfg.k1
    H0T = HALF_T                 # tiles in half 0
    H1T = nt - HALF_T
    NCHG_MAX = int(np.max(g_nch))

    nc = bacc.Bacc("TRN2", target_bir_lowering=False, debug=False,
                   num_devices=cfg.n_cores, num_swdge_queues=4)

    xt_d = nc.dram_tensor("xt", [cfg.in_dim, S], bf16, kind="ExternalInput")
    w1_d = nc.dram_tensor("w1", [128, cfg.kd * cfg.hid], bf16,
                          kind="ExternalInput")
    wfcT_d = nc.dram_tensor("wfcT", [cfg.hid, cfg.out_dim], bf16,
                            kind="ExternalInput")
    b1_d = nc.dram_tensor("b1", [cfg.hid, 1], f32, kind="ExternalInput")
    bfc_d = nc.dram_tensor("bfc", [cfg.out_dim, 1], f32, kind="ExternalInput")
    iota_d = nc.dram_tensor("iota", [128, 128], f32, kind="ExternalInput")
    partcol_d = nc.dram_tensor("partcol", [128, 1], f32, kind="ExternalInput")
    dinv_d = nc.dram_tensor("dinv_sb", [128, nt], f32, kind="ExternalInput")
    idx_d = nc.dram_tensor("idx", [128, n_idx_cols], i16, kind="ExternalInput")
    dstloc_d = nc.dram_tensor("dstloc", [128, n_chunks], f32,
                              kind="ExternalInput")
    dinvdst_d = nc.dram_tensor("dinvdst", [128, n_chunks], f32,
                               kind="ExternalInput")
    negdinvdst_d = nc.dram_tensor("negdinvdst", [128, n_chunks], f32,
                                  kind="ExternalInput")
    out_d = nc.dram_tensor("out", [cfg.out_dim, S], f32, kind="ExternalOutput")

    with tile.TileContext(nc) as tc, ExitStack() as ctx:
        dram = ctx.enter_context(tc.tile_pool(name="dram", bufs=1,
                                              space="DRAM"))
        consts = ctx.enter_context(tc.tile_pool(name="consts", bufs=1))
        persist = ctx.enter_context(tc.tile_pool(name="persist", bufs=1))
        ag0_in = dram.tile([H0T * 128, 128], bf16)
        ag1_in = dram.tile([H1T * 128, 128], bf16)
        ag0_out = dram.tile([cfg.half0_rows, 128], bf16, addr_space="Shared")
        ag1_out = dram.tile([cfg.half1_rows, 128], bf16, addr_space="Shared")

        # iota lives in PSUM: a PSUM-source tensor_scalar cannot enter the
        # 2-port DVE perf modes, so the per-chunk C-matrix builds never grab
        # the shared SBUF port pair that SWDGE descriptor generation (the
        # dma_gather Q7 loop) needs — they'd otherwise serialize.
        iota_pp = ctx.enter_context(
            tc.tile_pool(name="iotapp", bufs=1, space="PSUM"))
        iota_ps = iota_pp.tile([128, 128], f32)

        iota_sb = consts.tile([128, 128], f32)
        partcol_sb = consts.tile([128, 1], f32)
        w1_sb = consts.tile([128, cfg.kd * cfg.hid], bf16)
        wfcT_sb = consts.tile([cfg.hid, cfg.out_dim], bf16)
        b1_sb = consts.tile([cfg.hid, 1], f32)
        bfc_sb = consts.tile([cfg.out_dim, 1], f32)
        dinv_sb = consts.tile([128, nt], f32)
        idx_sb = consts.tile([128, n_idx_cols], i16)
        dstloc_sb = consts.tile([128, n_chunks], f32)
        dinvdst_sb = consts.tile([128, n_chunks], f32)
        negdinvdst_sb = consts.tile([128, n_chunks], f32)
        stage = persist.tile([128, nt, 128], bf16)
        xt_sb = persist.tile([128, cfg.kd, S], bf16)

        # xt first (phase 1's critical input), then the rest.
        nc.sync.dma_start(
            xt_sb[:, :, 0:H0T * 128],
            xt_d[:, 0:H0T * 128].rearrange("(k p) s -> p k s", p=128))
        nc.sync.dma_start(
            xt_sb[:, :, H0T * 128:S],
            xt_d[:, H0T * 128:S].rearrange("(k p) s -> p k s", p=128))
        nc.sync.dma_start(iota_sb[:], iota_d[:])
        nc.sync.dma_start(partcol_sb[:], partcol_d[:])
        nc.sync.dma_start(w1_sb[:], w1_d[:])
        nc.sync.dma_start(wfcT_sb[:], wfcT_d[:])
        nc.sync.dma_start(b1_sb[:], b1_d[:])
        nc.sync.dma_start(bfc_sb[:], bfc_d[:])
        nc.sync.dma_start(dinv_sb[:], dinv_d[:])
        nc.sync.dma_start(idx_sb[:], idx_d[:])
        nc.sync.dma_start(dstloc_sb[:], dstloc_d[:])
        nc.sync.dma_start(dinvdst_sb[:], dinvdst_d[:])
        nc.sync.dma_start(negdinvdst_sb[:], negdinvdst_d[:])
        nc.vector.tensor_scalar_mul(iota_ps[:], iota_sb[:], 1.0)

        # ---- Phase 1: h' = dinv * (X @ W1), bf16 rows padded to 256 B ----
        # Computed in two halves; each half is AllGather'd as soon as ready.
        with tc.tile_pool(name="p1ps", bufs=2, space="PSUM") as p1ps:
            for half, (t0, t1, ag_in, ag_out, rows) in enumerate((
                    (0, H0T, ag0_in, ag0_out, cfg.half0_rows),
                    (H0T, nt, ag1_in, ag1_out, cfg.half1_rows))):
                for t in range(t0, t1):
                    ph = p1ps.tile([128, cfg.hid], f32)
                    for k in range(cfg.kd):
                        nc.tensor.matmul(
                            ph[:],
                            xt_sb[:, k, t * 128:(t + 1) * 128],
                            w1_sb[:, k * cfg.hid:(k + 1) * cfg.hid],
                            start=(k == 0), stop=(k == cfg.kd - 1))
                    nc.vector.tensor_scalar_mul(
                        stage[:, t, 0:cfg.hid], ph[:],
                        dinv_sb[:, t:t + 1])
                nc.sync.dma_start(
                    ag_in[:].rearrange("(t p) e -> p t e", p=128),
                    stage[:, t0:t1, :])
                nc.gpsimd.collective_compute(
                    "AllGather",
                    mybir.AluOpType.bypass,
                    ins=[ag_in.opt()],
                    outs=[ag_out.opt()],
                    replica_groups=[list(range(cfg.n_cores))],
                )

        # ---- Phase 2: gather + one-hot scatter matmuls + layer 2 ----
        msgs_p = ctx.enter_context(tc.tile_pool(name="msgs", bufs=2))
        c_p = ctx.enter_context(tc.tile_pool(name="cmat", bufs=12))
        relu_p = ctx.enter_context(tc.tile_pool(name="relu", bufs=3))
        ost_p = ctx.enter_context(tc.tile_pool(name="ost", bufs=2))
        agg_ps = ctx.enter_context(
            tc.tile_pool(name="aggps", bufs=4, space="PSUM"))
        o2_ps = ctx.enter_context(
            tc.tile_pool(name="o2ps", bufs=2, space="PSUM"))

        GMAX = GATHER_MAX_CHUNKS
        gather_call = 0

        for g in range(cfg.n_groups):
            nchg = int(g_nch[g])
            nch0 = int(g_nch0[g])
            base = int(g_base[g])
            msgs = msgs_p.tile([128, NCHG_MAX, 128], bf16)
            col0 = base * 8
            for r0, r1, tbl in ((0, nch0, ag0_out[:, :]),
                                (nch0, nchg, ag1_out[:, :])):
                cs0 = r0
                while cs0 < r1:
                    nch = min(GMAX, r1 - cs0)
                    nc.gpsimd.dma_gather(
                        msgs[:, cs0:cs0 + nch, :], tbl,
                        idx_sb[:, col0 + cs0 * 8: col0 + (cs0 + nch) * 8],
                        nch * 128, nch * 128, 128,
                        single_packet=GATHER_SINGLE_PACKET,
                        queue_num=gather_call % 4)
                    gather_call += 1
                    cs0 += nch

            for i in range(G):
                p = g * G + i
                agg = agg_ps.tile([cfg.hid, 128], f32)
                # self-loop term first (no gather dependency): lhsT = stage
                # tile (dinv*h), rhs = diag(dinv).
                dmat = c_p.tile([128, 128], bf16)
                nc.vector.tensor_scalar(
                    dmat[:], iota_ps[:],
                    partcol_sb[:, 0:1],
                    dinv_sb[:, p:p + 1],
                    mybir.AluOpType.is_equal,
                    mybir.AluOpType.mult)
                nc.tensor.matmul(
                    agg[:], stage[:, p, 0:cfg.hid], dmat[:],
                    start=True, stop=False)
                slots = ([int(c0_in_g[p]) + j for j in range(int(k0[p]))]
                         + [int(c1_in_g[p]) + j for j in range(int(k1[p]))])
                for jj, cs in enumerate(slots):
                    gc = base + cs
                    cmat = c_p.tile([128, 128], bf16)
                    if gc % SCALAR_CBUILD_MOD == 0:
                        # ScalarE build: |dstloc - iota| -> relu(dinv*(1-t))
                        tabs = c_p.tile([128, 128], bf16)
                        nc.scalar.activation(
                            tabs[:], iota_sb[:],
                            mybir.ActivationFunctionType.Abs,
                            bias=dstloc_sb[:, gc:gc + 1], scale=-1.0)
                        nc.scalar.activation(
                            cmat[:], tabs[:],
                            mybir.ActivationFunctionType.Relu,
                            bias=dinvdst_sb[:, gc:gc + 1],
                            scale=negdinvdst_sb[:, gc:gc + 1])
                    else:
                        nc.vector.tensor_scalar(
                            cmat[:], iota_ps[:],
                            dstloc_sb[:, gc:gc + 1],
                            dinvdst_sb[:, gc:gc + 1],
                            mybir.AluOpType.is_equal,
                            mybir.AluOpType.mult)
                    nc.tensor.matmul(
                        agg[:], msgs[:, cs, 0:cfg.hid], cmat[:],
                        start=False, stop=(jj == len(slots) - 1))
                relu = relu_p.tile([cfg.hid, 128], bf16)
                nc.scalar.activation(
                    relu[:], agg[:], mybir.ActivationFunctionType.Relu,
                    bias=b1_sb[:])
                o2 = o2_ps.tile([cfg.out_dim, 128], f32)
                nc.tensor.matmul(o2[:], wfcT_sb[:], relu[:],
                                 start=True, stop=True)
                if i == 0:
                    ostage = ost_p.tile([cfg.out_dim, G * 128], f32)
                nc.scalar.activation(
                    ostage[:, i * 128:(i + 1) * 128], o2[:],
                    mybir.ActivationFunctionType.Identity, bias=bfc_sb[:])
            nc.sync.dma_start(
                out_d[:, g * G * 128:(g + 1) * G * 128], ostage[:])

    nc.compile()
    return nc


# ----------------------------------------------------------------------------
# Entry points.
# ----------------------------------------------------------------------------

_CACHE = {}


def _get_compiled(edges):
    import hashlib
    e = np.ascontiguousarray(np.asarray(edges, dtype=np.int64))
    key = (e.shape, hashlib.sha1(e.tobytes()).hexdigest())
    if key not in _CACHE:
        cfg, plan = _plan(e)
        nc = _build_module(cfg, plan)
        _CACHE[key] = (cfg, plan, nc)
    return _CACHE[key]


def _run(X, edges, W1, b1, Wfc, bfc, trace=False):
    from concourse.bass_utils import run_bass_kernel_spmd

    cfg, plan, nc = _get_compiled(edges)
    in_maps = _make_inputs(X, W1, b1, Wfc, bfc, cfg, plan)
    res = run_bass_kernel_spmd(
        nc, in_maps, core_ids=list(range(cfg.n_cores)), trace=trace)

    # Device output column (c, p, lane) -> node via stage_tile mapping.
    nt = cfg.nt
    full = np.concatenate([res.results[c]["out"] for c in range(cfg.n_cores)],
                          axis=1)                      # [40, n_slots]
    node_tile = plan["node_tile"]
    node_lane = plan["node_lane"]
    stage_tile = plan["stage_tile"]
    inv_stage = np.empty_like(stage_tile)
    for c in range(cfg.n_cores):
        inv_stage[c, stage_tile[c]] = np.arange(nt)
    n_core = node_tile // nt
    n_p = inv_stage[n_core, node_tile % nt]
    col = n_core * (nt * 128) + n_p * 128 + node_lane
    out = full[:, col].T.astype(np.float32)
    out = np.ascontiguousarray(out)
    return out, res


def kernel(X, edges, W1, b1, Wfc, bfc):
    out, _ = _run(np.asarray(X, dtype=np.float32), np.asarray(edges),
                  np.asarray(W1, dtype=np.float32),
                  np.asarray(b1, dtype=np.float32),
                  np.asarray(Wfc, dtype=np.float32),
                  np.asarray(bfc, dtype=np.float32))
    return out


# revision 23
# speedup vs baseline: 2.3902x; 1.0098x over previous
"""Trainium2 Bass kernel for a 2-layer GCN (GCNConv -> ReLU -> Linear).

Math (matching the PyG-style reference):
    deg  = in_degree(dst) + 1 (self loops), dinv = deg^-1/2
    h    = X @ W1                                  [N, 64]
    agg[d] = dinv[d] * sum_{e:(s->d)} dinv[s]*h[s] (+ self loop)   [N, 64]
    out  = relu(agg + b1) @ Wfc.T + bfc            [N, 40]

Distribution over 8 NeuronCores (graph/data parallel):
  - Nodes are re-labeled into 392 "tiles" of 128 slots (balanced so each
    (core, tile) has nearly equal lo/hi in-edge counts), 49 tiles per core.
    Each core computes h' = dinv*h for its 6272 slots (X @ W1 on the tensor
    engine) in two halves; each half is AllGather'd as soon as it is ready
    (two collectives, overlapped with compute / descriptor generation).
  - Each core aggregates the edges whose destination it owns: a bulk
    SWDGE dma_gather fetches h'[src] rows (256 B each) into SBUF in
    edge-major layout; destinations are scatter-added via one-hot matmuls
    (lhsT = gathered messages [128 edges, 64], rhs = C [128 edges, 128 dst]
    with C[j, d] = dinv_dst[j] * (dst_loc[j] == d)) accumulated in PSUM.
    C is built on the vector engine from a PSUM-resident iota (PSUM source
    keeps the op out of the 2-port DVE perf modes, which would otherwise
    lock GpSimd out of the shared SBUF port during SWDGE desc-gen).
  - Self loops never enter the gather stream: a per-tile diagonal matmul
    (lhsT = local h' tile, rhs = diag(dinv)) adds dinv^2*h directly.
  - relu(agg + b1) is fused into the PSUM->SBUF eviction on the scalar
    engine, the second layer is one small matmul per tile, and the bfc bias
    rides the final eviction.  The host un-permutes the [40, slots] outputs.

dma_gather indices are int16, so the table is built as TWO AllGather
outputs: half 0 = tiles 0..24 of every core (25600 rows), half 1 = tiles
25..48 (24576 rows); each tile's edges are packed into k0[t] half-0 chunks
followed by k1[t] half-1 chunks (pad lanes: idx=0, dst_loc=-1, dinv_dst=0).
The per-tile chunk-count profile (k0, k1) is shared by all cores (SPMD);
each core's tiles are assigned to profile slots by sorted edge counts.
"""

import numpy as np

# ----------------------------------------------------------------------------
# Problem configuration (hardcoded; kernel.py must be self-contained).
# ----------------------------------------------------------------------------
N_NODES = 50000
N_EDGES = 800000
IN_DIM = 512
HID = 64
OUT_DIM = 40
N_CORES = 8
TILES_PER_CORE = 49
GROUP = 7                    # tiles per gather/staging group
HALF_T = 17                  # tiles 0..HALF_T-1 are table half 0 (small half
                             # first: its AllGather completes early so class-0
                             # gathers start while the big half is still in
                             # flight; 32*8*128 = 32768 = int16 limit binds
                             # the big half)
SCALAR_CBUILD_MOD = 3        # every 3rd C-matrix build goes to ScalarE

GATHER_MAX_CHUNKS = 18
GATHER_SINGLE_PACKET = False


class Cfg:
    def __init__(self, n_nodes, in_dim, hid, out_dim, n_cores, k0, k1):
        self.n_nodes = n_nodes
        self.in_dim = in_dim
        self.hid = hid
        self.out_dim = out_dim
        self.n_cores = n_cores
        self.nt = TILES_PER_CORE
        self.group = GROUP
        self.n_groups = self.nt // self.group
        self.slots_per_core = self.nt * 128
        self.n_tiles = n_cores * self.nt
        self.n_slots = self.n_tiles * 128
        self.half0_rows = n_cores * HALF_T * 128       # 25600
        self.half1_rows = n_cores * (self.nt - HALF_T) * 128   # 24576
        self.k0 = k0                  # per-tile half-0 chunk counts [nt]
        self.k1 = k1                  # per-tile half-1 chunk counts [nt]
        self.n_chunks = int(np.sum(k0) + np.sum(k1))
        self.kd = in_dim // 128
        assert in_dim % 128 == 0
        assert self.half0_rows <= 32768 and self.half1_rows <= 32768


def _slot_of(tile, lane, n_cores):
    """Global table row of (global tile, lane) under the two-half layout."""
    core = tile // TILES_PER_CORE
    t = tile % TILES_PER_CORE
    half0 = n_cores * HALF_T * 128
    return np.where(
        t < HALF_T,
        core * (HALF_T * 128) + t * 128 + lane,
        half0 + core * ((TILES_PER_CORE - HALF_T) * 128) + (t - HALF_T) * 128
        + lane,
    )


# ----------------------------------------------------------------------------
# Host-side graph preprocessing (index/layout work only; all feature math
# runs on the device).
# ----------------------------------------------------------------------------

def _plan(edges):
    n_nodes, n_cores = N_NODES, N_CORES
    nt = TILES_PER_CORE
    n_tiles = n_cores * nt

    src = np.asarray(edges[0], dtype=np.int64)
    dst = np.asarray(edges[1], dtype=np.int64)
    deg = np.bincount(dst, minlength=n_nodes).astype(np.int64) + 1
    dinv = (1.0 / np.sqrt(deg.astype(np.float64))).astype(np.float32)

    # Snake binpack nodes (by degree, desc) into n_tiles bins of <=128 slots.
    order = np.argsort(-deg, kind="stable")
    rounds = np.arange(n_nodes) // n_tiles
    pos = np.arange(n_nodes) % n_tiles
    tile_of = np.where(rounds % 2 == 0, pos, n_tiles - 1 - pos)
    assert rounds.max() < 128, "more than 128 slots per tile"
    node_tile = np.empty(n_nodes, dtype=np.int64)
    node_lane = np.empty(n_nodes, dtype=np.int64)
    node_tile[order] = tile_of
    node_lane[order] = rounds

    half0 = n_cores * HALF_T * 128
    d_tile = node_tile[dst]
    d_lane = node_lane[dst]
    d_core = d_tile // nt
    d_tloc = d_tile % nt

    # The AllGather table stores each core's tiles in PROFILE-SLOT order
    # (stage column p holds local tile sort_t[c, p]), so an edge's class
    # (which table half its source lives in) depends on the source tile's
    # profile slot — which depends on per-class counts.  Two rounds: round 1
    # assigns profile slots using real-tile classes; round 2 recomputes
    # classes/counts under that fixed assignment.
    s_core = node_tile[src] // nt
    s_tloc = node_tile[src] % nt
    prof_slot = np.tile(np.arange(nt), (n_cores, 1))
    sort_t = np.tile(np.arange(nt), (n_cores, 1))
    for _round in range(2):
        s_p = prof_slot[s_core, s_tloc]
        cls = (s_p >= HALF_T).astype(np.int64)
        cnt = np.zeros((n_cores, nt, 2), dtype=np.int64)
        np.add.at(cnt, (d_core, d_tloc, cls), 1)
        key = cnt[:, :, 0] * 100000 + cnt[:, :, 1]
        sort_t = np.argsort(-key, axis=1, kind="stable")   # [cores, nt]
        prof_slot = np.empty((n_cores, nt), dtype=np.int64)
        for c in range(n_cores):
            prof_slot[c, sort_t[c]] = np.arange(nt)

    s_p = prof_slot[s_core, s_tloc]
    cls = (s_p >= HALF_T).astype(np.int64)
    cnt = np.zeros((n_cores, nt, 2), dtype=np.int64)
    np.add.at(cnt, (d_core, d_tloc, cls), 1)
    cnt_sorted = np.take_along_axis(cnt, sort_t[:, :, None], axis=1)
    k0 = np.max((cnt_sorted[:, :, 0] + 127) // 128, axis=0).astype(np.int64)
    k1 = np.max((cnt_sorted[:, :, 1] + 127) // 128, axis=0).astype(np.int64)
    k0 = np.maximum(k0, 1)
    k1 = np.maximum(k1, 1)
    # profile slots 0..HALF_T-1 must be the half-0 chunk-heavy ones?  No —
    # halves are POSITIONS: table half 0 = profile slots < HALF_T of every
    # core.  (sort order only balances counts.)

    cfg = Cfg(n_nodes, IN_DIM, HID, OUT_DIM, n_cores, k0, k1)

    # Source table row of each edge (profile-slot based).
    s_row = _slot_of(s_core * nt + s_p, node_lane[src], n_cores)

    slot_dinv = np.zeros(n_cores * nt * 128, dtype=np.float32)
    slot_dinv[_slot_of(node_tile, node_lane, n_cores)] = dinv

    # Chunk-slot numbering (shared across cores): profile slots are laid out
    # group-major; within a group: all half-0 chunks (tile-major), then all
    # half-1 chunks.
    #   base0[g] = start of group g's chunk range
    c0_in_g = np.zeros(nt, dtype=np.int64)   # chunk offset of tile within grp
    c1_in_g = np.zeros(nt, dtype=np.int64)
    g_nch = np.zeros(cfg.n_groups, dtype=np.int64)
    g_nch0 = np.zeros(cfg.n_groups, dtype=np.int64)
    g_base = np.zeros(cfg.n_groups, dtype=np.int64)
    acc = 0
    for g in range(cfg.n_groups):
        ts = np.arange(g * GROUP, (g + 1) * GROUP)
        g_base[g] = acc
        off = 0
        for t in ts:
            c0_in_g[t] = off
            off += k0[t]
        g_nch0[g] = off
        for t in ts:
            c1_in_g[t] = off
            off += k1[t]
        g_nch[g] = off
        acc += off
    n_chunks = acc
    assert n_chunks == cfg.n_chunks

    # Rank edges within (core, profile slot, class).
    p_slot = prof_slot[d_core, d_tloc]
    ekey = ((d_core * nt + p_slot) * 2 + cls)
    sort_idx = np.argsort(ekey, kind="stable")
    ekey_s = ekey[sort_idx]
    starts = np.searchsorted(ekey_s, np.arange(n_cores * nt * 2))
    rank = np.arange(len(ekey_s)) - starts[ekey_s]

    es_core = d_core[sort_idx]
    es_pslot = p_slot[sort_idx]
    es_cls = cls[sort_idx]
    es_g = es_pslot // GROUP
    j_chunk = rank >> 7
    lane = rank & 127
    fc = np.where(
        es_cls == 0,
        g_base[es_g] + c0_in_g[es_pslot] + j_chunk,
        g_base[es_g] + c1_in_g[es_pslot] + j_chunk,
    )
    assert (j_chunk < np.where(es_cls == 0, k0[es_pslot], k1[es_pslot])).all()

    idx16 = np.zeros((n_cores, n_chunks, 128), dtype=np.int16)
    dstloc = np.full((n_cores, n_chunks, 128), -1.0, dtype=np.float32)
    dinvdst = np.zeros((n_cores, n_chunks, 128), dtype=np.float32)

    s_rel = np.where(es_cls == 0, s_row[sort_idx],
                     s_row[sort_idx] - half0).astype(np.int16)
    idx16[es_core, fc, lane] = s_rel
    dstloc[es_core, fc, lane] = d_lane[sort_idx].astype(np.float32)
    dinvdst[es_core, fc, lane] = dinv[dst[sort_idx]]

    # Wrap gather indices: per (group, class) region, list position s ->
    # partition s%16, column s//16; replicated across the 8 q7 cores.
    n_idx_cols = n_chunks * 8
    idx_wrapped = np.zeros((n_cores, 128, n_idx_cols), dtype=np.int16)
    for g in range(cfg.n_groups):
        for lo, hi in ((0, g_nch0[g]), (g_nch0[g], g_nch[g])):
            nch = hi - lo
            if nch == 0:
                continue
            fc0 = g_base[g] + lo
            flat = idx16[:, fc0:fc0 + nch, :].reshape(n_cores, nch * 128)
            wrapped = flat.reshape(n_cores, nch * 8, 16).transpose(0, 2, 1)
            c0 = fc0 * 8
            idx_wrapped[:, :16, c0:c0 + nch * 8] = wrapped
    idx_wrapped[:, 16:, :] = np.tile(idx_wrapped[:, :16, :], (1, 7, 1))

    # Per-profile-slot dinv columns (for h' scaling and diag matmuls): the
    # lanes of (core, profile slot p) are those of its assigned local tile.
    # Phase-1 stage is laid out in PROFILE-SLOT order so that gather chunk /
    # matmul structure is SPMD-uniform; the AllGather table rows follow the
    # same order, and node_to_slot already accounts for it via _slot_of on
    # REAL tiles... stage column p holds local tile sort_t[c, p].
    stage_tile = sort_t                                   # [cores, nt]
    # dinv per (core, profile slot, lane):
    dinv_ps = np.zeros((n_cores, 128, nt), dtype=np.float32)
    for c in range(n_cores):
        for p in range(nt):
            t = stage_tile[c, p]
            gt = c * nt + t
            sl = _slot_of(np.full(128, gt), np.arange(128), n_cores)
            dinv_ps[c, :, p] = slot_dinv[sl]

    plan = dict(
        node_tile=node_tile, node_lane=node_lane,
        slot_dinv=slot_dinv, stage_tile=stage_tile,
        idx_wrapped=idx_wrapped,
        dstloc=dstloc.transpose(0, 2, 1).copy(),
        dinvdst=dinvdst.transpose(0, 2, 1).copy(),
        dinv_ps=dinv_ps,
        g_nch=g_nch, g_nch0=g_nch0, g_base=g_base,
        c0_in_g=c0_in_g, c1_in_g=c1_in_g,
    )
    return cfg, plan


def _make_inputs(X, W1, b1, Wfc, bfc, cfg, plan):
    """Build the 8 per-core input dicts for run_bass_kernel_spmd."""
    import ml_dtypes
    bf16 = ml_dtypes.bfloat16
    nt = cfg.nt

    # X rows arranged per (core, PROFILE slot, lane): stage column p of core
    # c holds local tile stage_tile[c, p].
    Xp = np.zeros((cfg.n_slots, cfg.in_dim), dtype=np.float32)
    # destination row for node n: core*nt*128 + prof_slot... we need X in the
    # ORDER phase 1 consumes it: xt[:, k, p*128+lane] = X[node at (c,p,lane)]
    node_tile = plan["node_tile"]
    node_lane = plan["node_lane"]
    stage_tile = plan["stage_tile"]
    inv_stage = np.empty_like(stage_tile)
    for c in range(cfg.n_cores):
        inv_stage[c, stage_tile[c]] = np.arange(nt)
    n_core = node_tile // nt
    n_tloc = node_tile % nt
    n_p = inv_stage[n_core, n_tloc]
    xrow = n_core * (nt * 128) + n_p * 128 + node_lane
    Xp[xrow] = np.asarray(X, dtype=np.float32)

    W1r = (np.asarray(W1, dtype=np.float32)
           .reshape(cfg.kd, 128, cfg.hid).transpose(1, 0, 2)
           .reshape(128, cfg.kd * cfg.hid).astype(bf16))
    wfcT = np.ascontiguousarray(np.asarray(Wfc, dtype=np.float32).T).astype(bf16)
    b1c = np.asarray(b1, dtype=np.float32).reshape(cfg.hid, 1)
    bfcc = np.asarray(bfc, dtype=np.float32).reshape(cfg.out_dim, 1)
    iota = np.ascontiguousarray(
        np.tile(np.arange(128, dtype=np.float32), (128, 1)))
    partcol = np.arange(128, dtype=np.float32).reshape(128, 1)

    s = cfg.slots_per_core
    in_maps = []
    for c in range(cfg.n_cores):
        xt = np.ascontiguousarray(Xp[c * s:(c + 1) * s].T).astype(bf16)
        in_maps.append({
            "xt": xt,
            "w1": W1r,
            "wfcT": wfcT,
            "b1": b1c,
            "bfc": bfcc,
            "iota": iota,
            "partcol": partcol,
            "dinv_sb": np.ascontiguousarray(plan["dinv_ps"][c]),
            "idx": plan["idx_wrapped"][c],
            "dstloc": plan["dstloc"][c],
            "dinvdst": plan["dinvdst"][c],
            "negdinvdst": np.ascontiguousarray(-plan["dinvdst"][c]),
        })
    return in_maps


# ----------------------------------------------------------------------------
# Device kernel.
# ----------------------------------------------------------------------------

def _build_module(cfg, plan):
    import concourse.bass as bass
    import concourse.bacc as bacc
    import concourse.mybir as mybir
    import concourse.tile as tile
    from contextlib import ExitStack

    f32 = mybir.dt.float32
    bf16 = mybir.dt.bfloat16
    i16 = mybir.dt.int16
    S = cfg.slots_per_core
    G = cfg.group
    nt = cfg.nt
    n_chunks = cfg.n_chunks
    n_idx_cols = n_chunks * 8
    g_nch = plan["g_nch"]
    g_nch0 = plan["g_nch0"]
    g_base = plan["g_base"]
    c0_in_g = plan["c0_in_g"]
    c1_in_g = plan["c1_in_g"]
    k0, k1 = cfg.k0, cfg.k1
    H0T = HALF_T                 # tiles in half 0
    H1T = nt - HALF_T
    NCHG_MAX = int(np.max(g_nch))

    nc = bacc.Bacc("TRN2", target_bir_lowering=False, debug=False,
                   num_devices=cfg.n_cores, num_swdge_queues=4)

    xt_d = nc.dram_tensor("xt", [cfg.in_dim, S], bf16, kind="ExternalInput")
    w1_d = nc.dram_tensor("w1", [128, cfg.kd * cfg.hid], bf16,
                          kind="ExternalInput")
    wfcT_d = nc.dram_tensor("wfcT", [cfg.hid, cfg.out_dim], bf16,
                            kind="ExternalInput")
    b1_d = nc.dram_tensor("b1", [cfg.hid, 1], f32, kind="ExternalInput")
    bfc_d = nc.dram_tensor("bfc", [cfg.out_dim, 1], f32, kind="ExternalInput")
    iota_d = nc.dram_tensor("iota", [128, 128], f32, kind="ExternalInput")
    partcol_d = nc.dram_tensor("partcol", [128, 1], f32, kind="ExternalInput")
    dinv_d = nc.dram_tensor("dinv_sb", [128, nt], f32, kind="ExternalInput")
    idx_d = nc.dram_tensor("idx", [128, n_idx_cols], i16, kind="ExternalInput")
    dstloc_d = nc.dram_tensor("dstloc", [128, n_chunks], f32,
                              kind="ExternalInput")
    dinvdst_d = nc.dram_tensor("dinvdst", [128, n_chunks], f32,
                               kind="ExternalInput")
    negdinvdst_d = nc.dram_tensor("negdinvdst", [128, n_chunks], f32,
                                  kind="ExternalInput")
    out_d = nc.dram_tensor("out", [cfg.out_dim, S], f32, kind="ExternalOutput")

    with tile.TileContext(nc) as tc, ExitStack() as ctx:
        dram = ctx.enter_context(tc.tile_pool(name="dram", bufs=1,
                                              space="DRAM"))
        consts = ctx.enter_context(tc.tile_pool(name="consts", bufs=1))
        persist = ctx.enter_context(tc.tile_pool(name="persist", bufs=1))
        ag0_in = dram.tile([H0T * 128, 128], bf16)
        ag1_in = dram.tile([H1T * 128, 128], bf16)
        ag0_out = dram.tile([cfg.half0_rows, 128], bf16, addr_space="Shared")
        ag1_out = dram.tile([cfg.half1_rows, 128], bf16, addr_space="Shared")

        # iota lives in PSUM: a PSUM-source tensor_scalar cannot enter the
        # 2-port DVE perf modes, so the per-chunk C-matrix builds never grab
        # the shared SBUF port pair that SWDGE descriptor generation (the
        # dma_gather Q7 loop) needs — they'd otherwise serialize.
        iota_pp = ctx.enter_context(
            tc.tile_pool(name="iotapp", bufs=1, space="PSUM"))
        iota_ps = iota_pp.tile([128, 128], f32)

        iota_sb = consts.tile([128, 128], f32)
        partcol_sb = consts.tile([128, 1], f32)
        w1_sb = consts.tile([128, cfg.kd * cfg.hid], bf16)
        wfcT_sb = consts.tile([cfg.hid, cfg.out_dim], bf16)
        b1_sb = consts.tile([cfg.hid, 1], f32)
        bfc_sb = consts.tile([cfg.out_dim, 1], f32)
        dinv_sb = consts.tile([128, nt], f32)
        idx_sb = consts.tile([128, n_idx_cols], i16)
        dstloc_sb = consts.tile([128, n_chunks], f32)
        dinvdst_sb = consts.tile([128, n_chunks], f32)
        negdinvdst_sb = consts.tile([128, n_chunks], f32)
        stage = persist.tile([128, nt, 128], bf16)
        xt_sb = persist.tile([128, cfg.kd, S], bf16)

        # Phase-1-critical inputs ride the sync HWDGE ring (kept clear so the
        # AllGather staging stores are not head-of-line blocked); the bulky
        # phase-2 constants (idx, dstloc, ...) go on the scalar HWDGE ring.
        nc.sync.dma_start(
            xt_sb[:, :, 0:H0T * 128],
            xt_d[:, 0:H0T * 128].rearrange("(k p) s -> p k s", p=128))
        nc.sync.dma_start(w1_sb[:], w1_d[:])
        nc.sync.dma_start(dinv_sb[:], dinv_d[:])
        nc.sync.dma_start(
            xt_sb[:, :, H0T * 128:S],
            xt_d[:, H0T * 128:S].rearrange("(k p) s -> p k s", p=128))
        nc.scalar.dma_start(iota_sb[:], iota_d[:])
        nc.scalar.dma_start(partcol_sb[:], partcol_d[:])
        nc.scalar.dma_start(wfcT_sb[:], wfcT_d[:])
        nc.scalar.dma_start(b1_sb[:], b1_d[:])
        nc.scalar.dma_start(bfc_sb[:], bfc_d[:])
        nc.scalar.dma_start(idx_sb[:], idx_d[:])
        nc.scalar.dma_start(dstloc_sb[:], dstloc_d[:])
        nc.scalar.dma_start(dinvdst_sb[:], dinvdst_d[:])
        nc.scalar.dma_start(negdinvdst_sb[:], negdinvdst_d[:])
        nc.vector.tensor_scalar_mul(iota_ps[:], iota_sb[:], 1.0)

        # ---- Phase 1: h' = dinv * (X @ W1), bf16 rows padded to 256 B ----
        # Computed in two halves; each half is AllGather'd as soon as ready.
        with tc.tile_pool(name="p1ps", bufs=2, space="PSUM") as p1ps:
            for half, (t0, t1, ag_in, ag_out, rows) in enumerate((
                    (0, H0T, ag0_in, ag0_out, cfg.half0_rows),
                    (H0T, nt, ag1_in, ag1_out, cfg.half1_rows))):
                for t in range(t0, t1):
                    ph = p1ps.tile([128, cfg.hid], f32)
                    for k in range(cfg.kd):
                        nc.tensor.matmul(
                            ph[:],
                            xt_sb[:, k, t * 128:(t + 1) * 128],
                            w1_sb[:, k * cfg.hid:(k + 1) * cfg.hid],
                            start=(k == 0), stop=(k == cfg.kd - 1))
                    nc.vector.tensor_scalar_mul(
                        stage[:, t, 0:cfg.hid], ph[:],
                        dinv_sb[:, t:t + 1])
                nc.sync.dma_start(
                    ag_in[:].rearrange("(t p) e -> p t e", p=128),
                    stage[:, t0:t1, :])
                nc.gpsimd.collective_compute(
                    "AllGather",
                    mybir.AluOpType.bypass,
                    ins=[ag_in.opt()],
                    outs=[ag_out.opt()],
                    replica_groups=[list(range(cfg.n_cores))],
                )

        # ---- Phase 2: gather + one-hot scatter matmuls + layer 2 ----
        msgs_p = ctx.enter_context(tc.tile_pool(name="msgs", bufs=2))
        c_p = ctx.enter_context(tc.tile_pool(name="cmat", bufs=12))
        relu_p = ctx.enter_context(tc.tile_pool(name="relu", bufs=3))
        ost_p = ctx.enter_context(tc.tile_pool(name="ost", bufs=2))
        agg_ps = ctx.enter_context(
            tc.tile_pool(name="aggps", bufs=4, space="PSUM"))
        o2_ps = ctx.enter_context(
            tc.tile_pool(name="o2ps", bufs=2, space="PSUM"))

        GMAX = GATHER_MAX_CHUNKS
        gather_call = 0

        for g in range(cfg.n_groups):
            nchg = int(g_nch[g])
            nch0 = int(g_nch0[g])
            base = int(g_base[g])
            msgs = msgs_p.tile([128, NCHG_MAX, 128], bf16)
            col0 = base * 8
            for r0, r1, tbl in ((0, nch0, ag0_out[:, :]),
                                (nch0, nchg, ag1_out[:, :])):
                cs0 = r0
                while cs0 < r1:
                    nch = min(GMAX, r1 - cs0)
                    nc.gpsimd.dma_gather(
                        msgs[:, cs0:cs0 + nch, :], tbl,
                        idx_sb[:, col0 + cs0 * 8: col0 + (cs0 + nch) * 8],
                        nch * 128, nch * 128, 128,
                        single_packet=GATHER_SINGLE_PACKET,
                        queue_num=gather_call % 4)
                    gather_call += 1
                    cs0 += nch

            for i in range(G):
                p = g * G + i
                agg = agg_ps.tile([cfg.hid, 128], f32)
                # self-loop term first (no gather dependency): lhsT = stage
                # tile (dinv*h), rhs = diag(dinv).
                dmat = c_p.tile([128, 128], bf16)
                nc.vector.tensor_scalar(
                    dmat[:], iota_ps[:],
                    partcol_sb[:, 0:1],
                    dinv_sb[:, p:p + 1],
                    mybir.AluOpType.is_equal,
                    mybir.AluOpType.mult)
                nc.tensor.matmul(
                    agg[:], stage[:, p, 0:cfg.hid], dmat[:],
                    start=True, stop=False)
                slots = ([int(c0_in_g[p]) + j for j in range(int(k0[p]))]
                         + [int(c1_in_g[p]) + j for j in range(int(k1[p]))])
                for jj, cs in enumerate(slots):
                    gc = base + cs
                    cmat = c_p.tile([128, 128], bf16)
                    if gc % SCALAR_CBUILD_MOD == 0:
                        # ScalarE build: |dstloc - iota| -> relu(dinv*(1-t))
                        tabs = c_p.tile([128, 128], bf16)
                        nc.scalar.activation(
                            tabs[:], iota_sb[:],
                            mybir.ActivationFunctionType.Abs,
                            bias=dstloc_sb[:, gc:gc + 1], scale=-1.0)
                        nc.scalar.activation(
                            cmat[:], tabs[:],
                            mybir.ActivationFunctionType.Relu,
                            bias=dinvdst_sb[:, gc:gc + 1],
                            scale=negdinvdst_sb[:, gc:gc + 1])
                    else:
                        nc.vector.tensor_scalar(
                            cmat[:], iota_ps[:],
                            dstloc_sb[:, gc:gc + 1],
                            dinvdst_sb[:, gc:gc + 1],
                            mybir.AluOpType.is_equal,
                            mybir.AluOpType.mult)
                    nc.tensor.matmul(
                        agg[:], msgs[:, cs, 0:cfg.hid], cmat[:],
                        start=False, stop=(jj == len(slots) - 1))
                relu = relu_p.tile([cfg.hid, 128], bf16)
                nc.scalar.activation(
                    relu[:], agg[:], mybir.ActivationFunctionType.Relu,
                    bias=b1_sb[:])
                o2 = o2_ps.tile([cfg.out_dim, 128], f32)
                nc.tensor.matmul(o2[:], wfcT_sb[:], relu[:],
                                 start=True, stop=True)
                if i == 0:
                    ostage = ost_p.tile([cfg.out_dim, G * 128], f32)
                nc.scalar.activation(
                    ostage[:, i * 128:(i + 1) * 128], o2[:],
                    mybir.ActivationFunctionType.Identity, bias=bfc_sb[:])
            nc.sync.dma_start(
                out_d[:, g * G * 128:(g + 1) * G * 128], ostage[:])

    nc.compile()
    return nc


# ----------------------------------------------------------------------------
# Entry points.
# ----------------------------------------------------------------------------

_CACHE = {}


def _get_compiled(edges):
    import hashlib
    e = np.ascontiguousarray(np.asarray(edges, dtype=np.int64))
    key = (e.shape, hashlib.sha1(e.tobytes()).hexdigest())
    if key not in _CACHE:
        cfg, plan = _plan(e)
        nc = _build_module(cfg, plan)
        _CACHE[key] = (cfg, plan, nc)
    return _CACHE[key]


def _run(X, edges, W1, b1, Wfc, bfc, trace=False):
    from concourse.bass_utils import run_bass_kernel_spmd

    cfg, plan, nc = _get_compiled(edges)
    in_maps = _make_inputs(X, W1, b1, Wfc, bfc, cfg, plan)
    res = run_bass_kernel_spmd(
        nc, in_maps, core_ids=list(range(cfg.n_cores)), trace=trace)

    # Device output column (c, p, lane) -> node via stage_tile mapping.
    nt = cfg.nt
    full = np.concatenate([res.results[c]["out"] for c in range(cfg.n_cores)],
                          axis=1)                      # [40, n_slots]
    node_tile = plan["node_tile"]
    node_lane = plan["node_lane"]
    stage_tile = plan["stage_tile"]
    inv_stage = np.empty_like(stage_tile)
    for c in range(cfg.n_cores):
        inv_stage[c, stage_tile[c]] = np.arange(nt)
    n_core = node_tile // nt
    n_p = inv_stage[n_core, node_tile % nt]
    col = n_core * (nt * 128) + n_p * 128 + node_lane
    out = full[:, col].T.astype(np.float32)
    out = np.ascontiguousarray(out)
    return out, res


def kernel(X, edges, W1, b1, Wfc, bfc):
    out, _ = _run(np.asarray(X, dtype=np.float32), np.asarray(edges),
                  np.asarray(W1, dtype=np.float32),
                  np.asarray(b1, dtype=np.float32),
                  np.asarray(Wfc, dtype=np.float32),
                  np.asarray(bfc, dtype=np.float32))
    return out
